# revision 5
# baseline (speedup 1.0000x reference)
"""GAT message-passing kernel for Trainium2, 8 NeuronCores — v2.

v2 changes over the baseline:
  - One-hot scatter matrices (oh) AND their transposes (ohT) are built on
    the host (layer-invariant), stored bf16 in DRAM, DMA-loaded per
    (layer, window).  The on-device DVE is_equal broadcast build (measured
    ~40ms of the 97ms baseline) is gone.
  - Per-edge a_dst is computed with tiny PE matmuls (lhsT=ohT, rhs=adst
    window column) instead of 5586 small indirect DMAs (~11ms).
  - The projection table [x_proj | a_src] is bf16: halves the AllGather
    and the per-edge gather traffic.  All big matmuls take bf16 inputs
    (1 cycle/row on PE vs 4 for fp32); PSUM accumulation stays fp32.
  - Residual stream, LayerNorms, softmax division stay fp32.
"""

import math
import os

import numpy as np
import ml_dtypes

import concourse.bass as bass
import concourse.mybir as mybir
import concourse.tile as tile
from concourse.bass_utils import run_bass_kernel_spmd

# problem dims (hardcoded per contract)
N, E, D, H, ED, L = 50000, 800000, 256, 8, 16, 6
C = D // H  # 32
DFF = 4 * D  # 1024
NEG_SLOPE = 0.2
EPS = 1e-5
NCORES = 8
P = 128

F32 = mybir.dt.float32
BF16 = mybir.dt.bfloat16
I32 = mybir.dt.int32
AX = mybir.AluOpType
AF = mybir.ActivationFunctionType

NEG_BIG = -1.0e30  # padded-edge a_edge => exp -> 0
NPBF = ml_dtypes.bfloat16


# ---------------------------------------------------------------------------
# host-side preprocessing
# ---------------------------------------------------------------------------

def _prep(inputs, n_nodes, n_edges, n_layers, n_cores):
    """Builds per-core input maps + the compile-time config."""
    x = np.asarray(inputs["x"], np.float32)
    edge_index = np.asarray(inputs["edge_index"])
    edge_attr = np.asarray(inputs["edge_attr"], np.float32)
    W = np.asarray(inputs["W"], np.float32)
    att_src = np.asarray(inputs["att_src"], np.float32)
    att_dst = np.asarray(inputs["att_dst"], np.float32)
    att_edge = np.asarray(inputs["att_edge"], np.float32)
    W_edge = np.asarray(inputs["W_edge"], np.float32)
    gat_bias = np.asarray(inputs["bias"], np.float32)

    d = W.shape[0]
    h = att_src.shape[0]
    c = att_src.shape[1]

    nsh = n_nodes // n_cores
    assert nsh * n_cores == n_nodes
    nw = math.ceil(nsh / P)
    lw = nsh - (nw - 1) * P  # rows in last window

    # fold attention vectors into the projection:  m @ W_ext ->
    # [x_proj | a_src | a_dst]
    S_src = np.zeros((d, h), np.float32)
    S_dst = np.zeros((d, h), np.float32)
    for hh in range(h):
        S_src[hh * c:(hh + 1) * c, hh] = att_src[hh]
        S_dst[hh * c:(hh + 1) * c, hh] = att_dst[hh]
    W_ext = np.concatenate([W, W @ S_src, W @ S_dst], axis=1)  # [d, d+2h]

    # per-edge a_edge = sum_c (ea @ W_edge)[h,c] * att_edge[h,c] = ea @ V
    V = np.zeros((W_edge.shape[0], h), np.float32)
    for hh in range(h):
        V[:, hh] = W_edge[:, hh * c:(hh + 1) * c] @ att_edge[hh]
    src0 = edge_index[0].astype(np.int64)
    dst0 = edge_index[1].astype(np.int64)

    # self-loop edge_attr = mean of incoming real-edge attrs (PyG default)
    order = np.argsort(dst0, kind="stable")
    dst_s = dst0[order]
    src_s = src0[order]
    ea_sum = np.zeros((n_nodes, W_edge.shape[0]), np.float32)
    if n_edges > 0:
        starts = np.flatnonzero(np.r_[True, dst_s[1:] != dst_s[:-1]])
        sums = np.add.reduceat(edge_attr[order], starts, axis=0)
        ea_sum[dst_s[starts]] = sums
    deg = np.bincount(dst0, minlength=n_nodes).astype(np.float32)
    ea_mean = ea_sum / np.maximum(deg, 1.0)[:, None]

    a_edge_real = (edge_attr @ V).astype(np.float32)[order]  # dst-sorted
    a_edge_self = (ea_mean @ V).astype(np.float32)

    # build per-core padded edge structure
    counts = np.zeros((n_cores, nw), np.int64)
    core_all = dst_s // nsh
    lw_all = (dst_s - core_all * nsh) // P
    np.add.at(counts, (core_all, lw_all), 1)
    node_ids = np.arange(n_nodes, dtype=np.int64)
    sc = node_ids // nsh
    slw = (node_ids - sc * nsh) // P
    np.add.at(counts, (sc, slw), 1)
    T = int(math.ceil(counts.max() / P))

    src_arr = np.zeros((n_cores, nw, P, T), np.int32)
    dl_arr = np.zeros((n_cores, nw, P, T), np.int32)
    aed_arr = np.full((n_cores, nw, P, T, h), NEG_BIG, np.float32)

    for cc in range(n_cores):
        base = cc * nsh
        lo = np.searchsorted(dst_s, base)
        hi = np.searchsorted(dst_s, base + nsh)
        cdst = dst_s[lo:hi] - base
        csrc = src_s[lo:hi]
        caed = a_edge_real[lo:hi]
        cw = cdst // P
        wstarts = np.searchsorted(cw, np.arange(nw))
        wends = np.searchsorted(cw, np.arange(nw) + 1)
        for w in range(nw):
            wsz = lw if w == nw - 1 else P
            s, e = wstarts[w], wends[w]
            self_nodes = base + w * P + np.arange(wsz)
            srcs = np.concatenate([csrc[s:e], self_nodes])
            dls = np.concatenate([cdst[s:e] - w * P,
                                  np.arange(wsz, dtype=np.int64)])
            aeds = np.concatenate([caed[s:e], a_edge_self[self_nodes]], axis=0)
            k = srcs.shape[0]
            assert k <= P * T
            pj = np.arange(k) % P
            tj = np.arange(k) // P
            src_arr[cc, w, pj, tj] = srcs
            dl_arr[cc, w, pj, tj] = dls
            aed_arr[cc, w, pj, tj] = aeds

    # ffn / ln weights
    w1 = np.asarray(inputs["ffn_w1"], np.float32)
    b1 = np.asarray(inputs["ffn_b1"], np.float32)
    w2 = np.asarray(inputs["ffn_w2"], np.float32)
    b2 = np.ascontiguousarray(np.asarray(inputs["ffn_b2"], np.float32))
    ln1g = np.ascontiguousarray(np.asarray(inputs["ln1_g"], np.float32))
    ln1b = np.ascontiguousarray(np.asarray(inputs["ln1_b"], np.float32))
    ln2g = np.ascontiguousarray(np.asarray(inputs["ln2_g"], np.float32))
    ln2b = np.ascontiguousarray(np.asarray(inputs["ln2_b"], np.float32))
    nmi = w1.shape[2] // P
    b1c = np.ascontiguousarray(
        b1.reshape(n_layers, nmi, P).transpose(0, 2, 1))

    # one-hot scatter matrices, bf16, layer-invariant
    # oh[cc, w, p_edge, t, n] = (dl[cc, w, p_edge, t] == n)
    eye = np.arange(P, dtype=np.int32)
    oh_all = (dl_arr[..., None] == eye).astype(NPBF)       # [cc,nw,P,T,P]
    ohT_all = np.ascontiguousarray(oh_all.transpose(0, 1, 4, 3, 2))

    w1b = np.ascontiguousarray(w1.astype(NPBF))
    w2b = np.ascontiguousarray(w2.astype(NPBF))
    wextb = np.ascontiguousarray(W_ext.astype(NPBF))
    aedb = np.ascontiguousarray(aed_arr.astype(NPBF))

    in_maps = []
    for cc in range(n_cores):
        in_maps.append({
            "x0": np.ascontiguousarray(x[cc * nsh:(cc + 1) * nsh]),
            "w_ext": wextb,
            "gat_bias": gat_bias.copy(),
            "w1_d": w1b, "b1c_d": b1c, "w2_d": w2b, "b2_d": b2,
            "ln1g_d": ln1g, "ln1b_d": ln1b,
            "ln2g_d": ln2g, "ln2b_d": ln2b,
            "src_d": np.ascontiguousarray(src_arr[cc]),
            "aed_d": aedb[cc],
            "oh_d": np.ascontiguousarray(oh_all[cc]),
            "ohT_d": ohT_all[cc],
        })
    cfg = dict(n_nodes=n_nodes, nsh=nsh, nw=nw, lw=lw, T=T,
               n_layers=n_layers, n_cores=n_cores, d=d, h=h,
               dff=w1.shape[2])
    return in_maps, cfg


# ---------------------------------------------------------------------------
# device kernel
# ---------------------------------------------------------------------------

def _legalize_single_wait(nc):
    """This walrus build allows at most one sync wait per instruction.

    Split extra waits onto standalone EventSemaphore instructions right
    before the owner (same engine => identical semantics).
    """
    def fix(blocks):
        n = 0
        for blk in blocks:
            newl = []
            for inst in list(blk.instructions):
                si = getattr(inst, "sync_info", None)
                ow = list(si.on_wait) if (si is not None and si.on_wait) else []
                if len(ow) > 1:
                    for j, wt in enumerate(ow[:-1]):
                        newl.append(mybir.InstEventSemaphore(
                            name=f"{inst.name}wf{j}",
                            sync_info=mybir.SyncInfo(on_wait=[wt],
                                                     on_update=[]),
                            engine=inst.engine,
                        ))
                    inst.sync_info = mybir.SyncInfo(
                        on_wait=[ow[-1]], on_update=list(si.on_update))
                    n += 1
                newl.append(inst)
            blk.instructions = newl
            subs = list(blk.blocks) if getattr(blk, "blocks", None) else []
            if subs:
                n += fix(subs)
        return n

    for f in nc.m.functions:
        fix(list(f.blocks))


def build_nc(cfg, legalize=True):
    n_nodes = cfg["n_nodes"]
    nsh = cfg["nsh"]
    NW = cfg["nw"]
    LW = cfg["lw"]
    T = cfg["T"]
    LAYERS = cfg["n_layers"]
    n_cores = cfg["n_cores"]
    d = cfg["d"]
    h = cfg["h"]
    dff = cfg["dff"]
    KT = d // P          # K-tiles for D-contraction (2)
    NMI = dff // P       # M-tiles for dff (8)
    TBL = d + h          # 264: x_proj | a_src
    EXT = d + 2 * h      # 272: x_proj | a_src | a_dst

    abl = set((os.environ.get("GAT_ABL") or "").split(","))

    nc = bass.Bass("TRN2", target_bir_lowering=False, debug=False,
                   num_devices=n_cores,
                   dynamic_dma_scratch_size=int(
                       os.environ.get("GAT_DMASCRATCH", "16384")))

    x0 = nc.dram_tensor("x0", [nsh, d], F32, kind="ExternalInput")
    w_ext = nc.dram_tensor("w_ext", [d, EXT], BF16, kind="ExternalInput")
    gat_bias = nc.dram_tensor("gat_bias", [d], F32, kind="ExternalInput")
    w1_d = nc.dram_tensor("w1_d", [LAYERS, d, dff], BF16,
                          kind="ExternalInput")
    b1c_d = nc.dram_tensor("b1c_d", [LAYERS, P, NMI], F32,
                           kind="ExternalInput")
    w2_d = nc.dram_tensor("w2_d", [LAYERS, dff, d], BF16,
                          kind="ExternalInput")
    b2_d = nc.dram_tensor("b2_d", [LAYERS, d], F32, kind="ExternalInput")
    ln1g_d = nc.dram_tensor("ln1g_d", [LAYERS, d], F32, kind="ExternalInput")
    ln1b_d = nc.dram_tensor("ln1b_d", [LAYERS, d], F32, kind="ExternalInput")
    ln2g_d = nc.dram_tensor("ln2g_d", [LAYERS, d], F32, kind="ExternalInput")
    ln2b_d = nc.dram_tensor("ln2b_d", [LAYERS, d], F32, kind="ExternalInput")
    src_d = nc.dram_tensor("src_d", [NW, P, T], I32, kind="ExternalInput")
    aed_d = nc.dram_tensor("aed_d", [NW, P, T, h], BF16,
                           kind="ExternalInput")
    oh_d = nc.dram_tensor("oh_d", [NW, P, T, P], BF16, kind="ExternalInput")
    ohT_d = nc.dram_tensor("ohT_d", [NW, P, T, P], BF16,
                           kind="ExternalInput")
    out_d = nc.dram_tensor("out_d", [nsh, d], F32, kind="ExternalOutput")

    G = min(7, NW)  # windows per group (epilogue/FFN batching granularity)
    groups = [(g0, min(g0 + G, NW) - g0) for g0 in range(0, NW, G)]

    def wsz(w):
        return LW if w == NW - 1 else P

    with tile.TileContext(nc) as tc:
        with (
            tc.tile_pool(name="const", bufs=1) as const,
            tc.tile_pool(name="wpool", bufs=1) as wpool,
            tc.tile_pool(name="big", bufs=1) as big,
            tc.tile_pool(name="work", bufs=2) as work,
            tc.tile_pool(name="edge", bufs=2) as edge,
            tc.tile_pool(name="small", bufs=2) as small,
            tc.tile_pool(name="stats", bufs=1) as stats,
            tc.tile_pool(name="psum", bufs=1, space="PSUM") as psum,
            tc.tile_pool(name="dram", bufs=1, space="DRAM") as dram,
        ):
            # ---------- constants ----------
            if "reset" in (os.environ.get("GAT_OPT") or ""):
                nc.gpsimd.dma_reset()
            from concourse.masks import make_identity
            identf = const.tile([P, P], F32)
            make_identity(nc, identf[:, :])
            wext_sb = const.tile([P, KT, EXT], BF16)
            nc.sync.dma_start(
                out=wext_sb[:, :, :],
                in_=w_ext[:, :].rearrange("(kk p) c -> p kk c", p=P))
            bias_b = const.tile([P, d], F32)
            nc.sync.dma_start(out=bias_b[:, :],
                              in_=gat_bias[None, :].to_broadcast((P, d)))
            eps_t = const.tile([P, 1], F32)
            nc.vector.memset(eps_t[:, :], EPS)

            # ---------- persistent node state ----------
            m_sb = big.tile([P, NW, d], F32)
            if LW < P:
                nc.vector.memset(m_sb[:, NW - 1, :], 0.0)
            for w in range(NW):
                nc.sync.dma_start(out=m_sb[:wsz(w), w, :],
                                  in_=x0[w * P:w * P + wsz(w), :])
            adst_sb = big.tile([P, NW, h], BF16)

            REP = int(os.environ.get("GAT_REPEAT", "1"))
            for layer in [ly for _ in range(REP) for ly in range(LAYERS)]:
                # ---------- per-layer weights ----------
                w1_sb = wpool.tile([P, KT, dff], BF16, tag="w1")
                nc.sync.dma_start(
                    out=w1_sb[:, :, :],
                    in_=w1_d[layer, :, :].rearrange("(kk p) f -> p kk f", p=P))
                w2_sb = wpool.tile([P, NMI, d], BF16, tag="w2")
                nc.sync.dma_start(
                    out=w2_sb[:, :, :],
                    in_=w2_d[layer, :, :].rearrange("(kk p) f -> p kk f", p=P))
                b1c_sb = wpool.tile([P, NMI], F32, tag="b1c")
                nc.sync.dma_start(out=b1c_sb[:, :], in_=b1c_d[layer, :, :])
                b2_b = wpool.tile([P, d], F32, tag="b2")
                nc.sync.dma_start(
                    out=b2_b[:, :],
                    in_=b2_d[layer:layer + 1, :].to_broadcast((P, d)))
                ln1g_b = wpool.tile([P, d], F32, tag="ln1g")
                nc.sync.dma_start(
                    out=ln1g_b[:, :],
                    in_=ln1g_d[layer:layer + 1, :].to_broadcast((P, d)))
                ln1b_b = wpool.tile([P, d], F32, tag="ln1b")
                nc.sync.dma_start(
                    out=ln1b_b[:, :],
                    in_=ln1b_d[layer:layer + 1, :].to_broadcast((P, d)))
                ln2g_b = wpool.tile([P, d], F32, tag="ln2g")
                nc.sync.dma_start(
                    out=ln2g_b[:, :],
                    in_=ln2g_d[layer:layer + 1, :].to_broadcast((P, d)))
                ln2b_b = wpool.tile([P, d], F32, tag="ln2b")
                nc.sync.dma_start(
                    out=ln2b_b[:, :],
                    in_=ln2b_d[layer:layer + 1, :].to_broadcast((P, d)))

                shard_t = dram.tile([nsh, TBL], BF16, tag="shard")
                table_t = dram.tile([n_nodes, TBL], BF16, tag="table",
                                    addr_space="Shared")

                # ---------- phase A: x_proj | a_src | a_dst ----------
                for w in range(NW):
                    mT_sb = work.tile([P, KT, P], BF16, tag="mT")
                    for kk in range(KT):
                        tp = psum.tile([P, P], F32, tag="t1", bufs=2)
                        nc.tensor.transpose(
                            tp[:, :], m_sb[:, w, kk * P:(kk + 1) * P],
                            identf[:, :])
                        nc.scalar.activation(mT_sb[:, kk, :], tp[:, :],
                                             AF.Copy)
                    pj = psum.tile([P, EXT], F32, tag="t2", bufs=2)
                    for kk in range(KT):
                        nc.tensor.matmul(pj[:, :], lhsT=mT_sb[:, kk, :],
                                         rhs=wext_sb[:, kk, :],
                                         start=(kk == 0), stop=(kk == KT - 1))
                    prj = work.tile([P, EXT], BF16, tag="prj")
                    nc.scalar.activation(prj[:, :], pj[:, :], AF.Copy)
                    nc.sync.dma_start(out=shard_t[w * P:w * P + wsz(w), :],
                                      in_=prj[:wsz(w), :TBL])
                    nc.vector.tensor_copy(adst_sb[:, w, :], prj[:, TBL:EXT])

                # ---------- all-gather the projection table ----------
                if "noag" not in abl:
                    nc.gpsimd.collective_compute(
                        "AllGather",
                        AX.bypass,
                        replica_groups=[list(range(n_cores))],
                        ins=[shard_t.opt()],
                        outs=[table_t.opt()],
                    )

                # ---------- grouped phases B (edges) + C (FFN) ----------
                for g0, gn in groups:
                    gsl = slice(g0, g0 + gn)
                    g_grp = work.tile([P, G, TBL], F32, tag="ggrp")
                    # group metadata in one DMA each
                    idx_g = small.tile([P, G, T], I32, tag="idx")
                    nc.sync.dma_start(out=idx_g[:, :gn, :],
                                      in_=src_d[gsl, :, :].rearrange(
                                          "g p t -> p g t"))
                    aed_g = work.tile([P, G, T, h], BF16, tag="aed")
                    nc.sync.dma_start(out=aed_g[:, :gn, :, :],
                                      in_=aed_d[gsl, :, :, :].rearrange(
                                          "g p t hh -> p g t hh"))
                    for wi in range(gn):
                        w = g0 + wi
                        oh_w = edge.tile([P, T, P], BF16, tag="oh")
                        nc.sync.dma_start(out=oh_w[:, :, :],
                                          in_=oh_d[w, :, :, :])
                        ohT_w = edge.tile([P, T, P], BF16, tag="ohT")
                        nc.sync.dma_start(out=ohT_w[:, :, :],
                                          in_=ohT_d[w, :, :, :])
                        gat_w = edge.tile([P, T, TBL], BF16, tag="gat")
                        if "nogat" in abl:
                            nc.vector.memset(gat_w[:, 0, :], 0.5)
                        else:
                            for t in range(T):
                                nc.gpsimd.indirect_dma_start(
                                    out=gat_w[:, t, :],
                                    out_offset=None,
                                    in_=table_t[:, :],
                                    in_offset=bass.IndirectOffsetOnAxis(
                                        ap=idx_g[:, wi, t:t + 1], axis=0),
                                )
                        # per-edge a_dst via tiny matmuls: [edge,h] per t
                        adxp = psum.tile([P, T * h], F32, tag="t3", bufs=2)
                        for t in range(T):
                            nc.tensor.matmul(adxp[:, t * h:(t + 1) * h],
                                             lhsT=ohT_w[:, t, :],
                                             rhs=adst_sb[:, w, :],
                                             start=True, stop=True)
                        adx_w = small.tile([P, T, h], BF16, tag="adx")
                        nc.scalar.activation(
                            adx_w[:, :, :],
                            adxp[:, :].rearrange("p (t hh) -> p t hh", hh=h),
                            AF.Copy)
                        # alpha = a_src[src]+a_dst[dst]+a_edge; lrelu; exp
                        alpha = small.tile([P, T, h], BF16, tag="alpha")
                        nc.vector.tensor_tensor(out=alpha[:, :, :],
                                                in0=aed_g[:, wi, :, :],
                                                in1=gat_w[:, :, d:TBL],
                                                op=AX.add)
                        nc.vector.tensor_tensor(out=alpha[:, :, :],
                                                in0=alpha[:, :, :],
                                                in1=adx_w[:, :, :],
                                                op=AX.add)
                        nc.vector.tensor_scalar(out=adx_w[:, :, :],
                                                in0=alpha[:, :, :],
                                                scalar1=NEG_SLOPE,
                                                scalar2=None, op0=AX.mult)
                        nc.vector.tensor_tensor(out=alpha[:, :, :],
                                                in0=alpha[:, :, :],
                                                in1=adx_w[:, :, :],
                                                op=AX.max)
                        nc.scalar.activation(gat_w[:, :, d:TBL],
                                             alpha[:, :, :], AF.Exp)
                        # messages: x_proj *= ex (broadcast over channels)
                        nc.vector.tensor_tensor(
                            out=gat_w[:, :, 0:d].rearrange(
                                "p t (hh c) -> p t hh c", c=C),
                            in0=gat_w[:, :, 0:d].rearrange(
                                "p t (hh c) -> p t hh c", c=C),
                            in1=gat_w[:, :, d:TBL][:, :, :, None].to_broadcast(
                                (P, T, h, C)),
                            op=AX.mult)
                        # accumulate  [window, x_sum | ex_sum]
                        acc = psum.tile([P, TBL], F32, tag="t2", bufs=2)
                        nts = 1 if "noacc" in abl else T
                        for t in range(nts):
                            nc.tensor.matmul(acc[:, :], lhsT=oh_w[:, t, :],
                                             rhs=gat_w[:, t, :],
                                             start=(t == 0),
                                             stop=(t == nts - 1))
                        nc.scalar.activation(g_grp[:, wi, :], acc[:, :],
                                             AF.Copy)

                    # ---- group epilogue: softmax div + bias + resid + LN1
                    den = g_grp[:, :gn, d:TBL]
                    nc.vector.tensor_scalar(out=den, in0=den, scalar1=1e-30,
                                            scalar2=None, op0=AX.max)
                    rec = stats.tile([P, G, h], F32, tag="rec", bufs=2)
                    nc.vector.reciprocal(rec[:, :gn, :], den)
                    nc.vector.tensor_tensor(
                        out=g_grp[:, :gn, 0:d].rearrange(
                            "p w (hh c) -> p w hh c", c=C),
                        in0=g_grp[:, :gn, 0:d].rearrange(
                            "p w (hh c) -> p w hh c", c=C),
                        in1=rec[:, :gn, :, None].to_broadcast((P, gn, h, C)),
                        op=AX.mult)
                    nc.vector.tensor_tensor(
                        out=g_grp[:, :gn, 0:d], in0=g_grp[:, :gn, 0:d],
                        in1=bias_b[:, None, :].to_broadcast((P, gn, d)),
                        op=AX.add)
                    nc.vector.tensor_tensor(out=m_sb[:, gsl, :],
                                            in0=m_sb[:, gsl, :],
                                            in1=g_grp[:, :gn, 0:d],
                                            op=AX.add)
                    _layernorm_batched(nc, stats, m_sb[:, gsl, :],
                                       g_grp[:, :gn, 0:d], ln1g_b, ln1b_b,
                                       eps_t, gn, d)

                    # ---- FFN for the group's windows
                    for wi in range(gn):
                        w = g0 + wi
                        mT_sb = work.tile([P, KT, P], BF16, tag="mT")
                        for kk in range(KT):
                            tp = psum.tile([P, P], F32, tag="t1", bufs=2)
                            nc.tensor.transpose(
                                tp[:, :], m_sb[:, w, kk * P:(kk + 1) * P],
                                identf[:, :])
                            nc.scalar.activation(mT_sb[:, kk, :], tp[:, :],
                                                 AF.Copy)
                        h1T = work.tile([P, NMI, P], BF16, tag="h1T")
                        half = NMI // 2
                        for hh2 in range(2):
                            tag = "t2" if hh2 == 0 else "t3"
                            hp = psum.tile([P, half, P], F32, tag=tag, bufs=2)
                            for sl in range(half):
                                mi = hh2 * half + sl
                                for kk in range(KT):
                                    nc.tensor.matmul(
                                        hp[:, sl, :],
                                        lhsT=w1_sb[:, kk, mi * P:(mi + 1) * P],
                                        rhs=mT_sb[:, kk, :],
                                        start=(kk == 0), stop=(kk == KT - 1))
                            nc.vector.tensor_tensor(
                                out=h1T[:, hh2 * half:(hh2 + 1) * half, :],
                                in0=hp[:, :, :],
                                in1=b1c_sb[:, hh2 * half:(hh2 + 1) * half,
                                           None].to_broadcast((P, half, P)),
                                op=AX.add)
                        nc.scalar.activation(h1T[:, :, :], h1T[:, :, :],
                                             AF.Relu)
                        h2p = psum.tile([P, d], F32, tag="t1", bufs=2)
                        for mi in range(NMI):
                            nc.tensor.matmul(h2p[:, :], lhsT=h1T[:, mi, :],
                                             rhs=w2_sb[:, mi, :],
                                             start=(mi == 0),
                                             stop=(mi == NMI - 1))
                        nc.scalar.activation(g_grp[:, wi, 0:d], h2p[:, :],
                                             AF.Copy)

                    # ---- h + b2 + resid + LN2
                    nc.vector.tensor_tensor(
                        out=g_grp[:, :gn, 0:d], in0=g_grp[:, :gn, 0:d],
                        in1=b2_b[:, None, :].to_broadcast((P, gn, d)),
                        op=AX.add)
                    nc.vector.tensor_tensor(out=m_sb[:, gsl, :],
                                            in0=m_sb[:, gsl, :],
                                            in1=g_grp[:, :gn, 0:d],
                                            op=AX.add)
                    _layernorm_batched(nc, stats, m_sb[:, gsl, :],
                                       g_grp[:, :gn, 0:d], ln2g_b, ln2b_b,
                                       eps_t, gn, d)

            # ---------- output ----------
            for w in range(NW):
                nc.sync.dma_start(out=out_d[w * P:w * P + wsz(w), :],
                                  in_=m_sb[:wsz(w), w, :])
    if legalize:
        _legalize_single_wait(nc)
    return nc


def _layernorm_batched(nc, stats, m_ap, scratch_ap, g_b, b_b, eps_t, nwin, d):
    """In-place LayerNorm over the feature axis of m_ap [P, nwin, d]."""
    ssum = stats.tile([P, nwin], F32, tag="ssum", bufs=2)
    nc.vector.tensor_reduce(out=ssum[:, :], in_=m_ap,
                            axis=mybir.AxisListType.X, op=AX.add)
    nc.vector.tensor_tensor(out=scratch_ap, in0=m_ap, in1=m_ap, op=AX.mult)
    ssq = stats.tile([P, nwin], F32, tag="ssq", bufs=2)
    nc.vector.tensor_reduce(out=ssq[:, :], in_=scratch_ap,
                            axis=mybir.AxisListType.X, op=AX.add)
    mu = stats.tile([P, nwin], F32, tag="mu", bufs=2)
    nc.vector.tensor_scalar(out=mu[:, :], in0=ssum[:, :], scalar1=1.0 / d,
                            scalar2=None, op0=AX.mult)
    var = stats.tile([P, nwin], F32, tag="var", bufs=2)
    nc.vector.tensor_scalar(out=var[:, :], in0=ssq[:, :], scalar1=1.0 / d,
                            scalar2=None, op0=AX.mult)
    mu2 = stats.tile([P, nwin], F32, tag="mu2", bufs=2)
    nc.vector.tensor_tensor(out=mu2[:, :], in0=mu[:, :], in1=mu[:, :],
                            op=AX.mult)
    nc.vector.tensor_tensor(out=var[:, :], in0=var[:, :], in1=mu2[:, :],
                            op=AX.subtract)
    nc.scalar.activation(var[:, :], var[:, :], AF.Sqrt, bias=eps_t[:, :1])
    rstd = stats.tile([P, nwin], F32, tag="rstd", bufs=2)
    nc.vector.reciprocal(rstd[:, :], var[:, :])
    nc.vector.tensor_tensor(out=m_ap, in0=m_ap,
                            in1=mu[:, :, None].to_broadcast((P, nwin, d)),
                            op=AX.subtract)
    nc.vector.tensor_tensor(out=m_ap, in0=m_ap,
                            in1=rstd[:, :, None].to_broadcast((P, nwin, d)),
                            op=AX.mult)
    nc.vector.tensor_tensor(out=m_ap, in0=m_ap,
                            in1=g_b[:, None, :].to_broadcast((P, nwin, d)),
                            op=AX.mult)
    nc.vector.tensor_tensor(out=m_ap, in0=m_ap,
                            in1=b_b[:, None, :].to_broadcast((P, nwin, d)),
                            op=AX.add)


# ---------------------------------------------------------------------------
# entry point
# ---------------------------------------------------------------------------

_LAST_RESULTS = {}


def kernel(**inputs):
    n_nodes = inputs["x"].shape[0]
    n_edges = inputs["edge_index"].shape[1]
    n_layers = inputs["ffn_w1"].shape[0]
    in_maps, cfg = _prep(inputs, n_nodes, n_edges, n_layers, NCORES)
    nc = build_nc(cfg)
    res = run_bass_kernel_spmd(
        nc, in_maps, list(range(NCORES)),
        trace=bool(int(os.environ.get("GAT_TRACE", "0"))),
    )
    _LAST_RESULTS["res"] = res
    out = np.concatenate([res.results[cc]["out_d"] for cc in range(NCORES)],
                         axis=0)
    return out


# revision 6
# speedup vs baseline: 1.0314x; 1.0314x over previous
"""GAT message-passing kernel for Trainium2, 8 NeuronCores — v2.

v2 changes over the baseline:
  - One-hot scatter matrices (oh) AND their transposes (ohT) are built on
    the host (layer-invariant), stored bf16 in DRAM, DMA-loaded per
    (layer, window).  The on-device DVE is_equal broadcast build (measured
    ~40ms of the 97ms baseline) is gone.
  - Per-edge a_dst is computed with tiny PE matmuls (lhsT=ohT, rhs=adst
    window column) instead of 5586 small indirect DMAs (~11ms).
  - The projection table [x_proj | a_src] is bf16: halves the AllGather
    and the per-edge gather traffic.  All big matmuls take bf16 inputs
    (1 cycle/row on PE vs 4 for fp32); PSUM accumulation stays fp32.
  - Residual stream, LayerNorms, softmax division stay fp32.
"""

import math
import os

import numpy as np
import ml_dtypes

import concourse.bass as bass
import concourse.mybir as mybir
import concourse.tile as tile
from concourse.bass_utils import run_bass_kernel_spmd

# problem dims (hardcoded per contract)
N, E, D, H, ED, L = 50000, 800000, 256, 8, 16, 6
C = D // H  # 32
DFF = 4 * D  # 1024
NEG_SLOPE = 0.2
EPS = 1e-5
NCORES = 8
P = 128

F32 = mybir.dt.float32
BF16 = mybir.dt.bfloat16
I32 = mybir.dt.int32
AX = mybir.AluOpType
AF = mybir.ActivationFunctionType

NEG_BIG = -1.0e30  # padded-edge a_edge => exp -> 0
NPBF = ml_dtypes.bfloat16


# ---------------------------------------------------------------------------
# host-side preprocessing
# ---------------------------------------------------------------------------

def _prep(inputs, n_nodes, n_edges, n_layers, n_cores):
    """Builds per-core input maps + the compile-time config."""
    x = np.asarray(inputs["x"], np.float32)
    edge_index = np.asarray(inputs["edge_index"])
    edge_attr = np.asarray(inputs["edge_attr"], np.float32)
    W = np.asarray(inputs["W"], np.float32)
    att_src = np.asarray(inputs["att_src"], np.float32)
    att_dst = np.asarray(inputs["att_dst"], np.float32)
    att_edge = np.asarray(inputs["att_edge"], np.float32)
    W_edge = np.asarray(inputs["W_edge"], np.float32)
    gat_bias = np.asarray(inputs["bias"], np.float32)

    d = W.shape[0]
    h = att_src.shape[0]
    c = att_src.shape[1]

    nsh = n_nodes // n_cores
    assert nsh * n_cores == n_nodes
    nw = math.ceil(nsh / P)
    lw = nsh - (nw - 1) * P  # rows in last window

    # fold attention vectors into the projection:  m @ W_ext ->
    # [x_proj | a_src | a_dst]
    S_src = np.zeros((d, h), np.float32)
    S_dst = np.zeros((d, h), np.float32)
    for hh in range(h):
        S_src[hh * c:(hh + 1) * c, hh] = att_src[hh]
        S_dst[hh * c:(hh + 1) * c, hh] = att_dst[hh]
    W_ext = np.concatenate([W, W @ S_src, W @ S_dst], axis=1)  # [d, d+2h]

    # per-edge a_edge = sum_c (ea @ W_edge)[h,c] * att_edge[h,c] = ea @ V
    V = np.zeros((W_edge.shape[0], h), np.float32)
    for hh in range(h):
        V[:, hh] = W_edge[:, hh * c:(hh + 1) * c] @ att_edge[hh]
    src0 = edge_index[0].astype(np.int64)
    dst0 = edge_index[1].astype(np.int64)

    # self-loop edge_attr = mean of incoming real-edge attrs (PyG default)
    order = np.argsort(dst0, kind="stable")
    dst_s = dst0[order]
    src_s = src0[order]
    ea_sum = np.zeros((n_nodes, W_edge.shape[0]), np.float32)
    if n_edges > 0:
        starts = np.flatnonzero(np.r_[True, dst_s[1:] != dst_s[:-1]])
        sums = np.add.reduceat(edge_attr[order], starts, axis=0)
        ea_sum[dst_s[starts]] = sums
    deg = np.bincount(dst0, minlength=n_nodes).astype(np.float32)
    ea_mean = ea_sum / np.maximum(deg, 1.0)[:, None]

    a_edge_real = (edge_attr @ V).astype(np.float32)[order]  # dst-sorted
    a_edge_self = (ea_mean @ V).astype(np.float32)

    # build per-core padded edge structure
    counts = np.zeros((n_cores, nw), np.int64)
    core_all = dst_s // nsh
    lw_all = (dst_s - core_all * nsh) // P
    np.add.at(counts, (core_all, lw_all), 1)
    node_ids = np.arange(n_nodes, dtype=np.int64)
    sc = node_ids // nsh
    slw = (node_ids - sc * nsh) // P
    np.add.at(counts, (sc, slw), 1)
    T = int(math.ceil(counts.max() / P))

    src_arr = np.zeros((n_cores, nw, P, T), np.int32)
    dl_arr = np.zeros((n_cores, nw, P, T), np.int32)
    aed_arr = np.full((n_cores, nw, P, T, h), NEG_BIG, np.float32)

    for cc in range(n_cores):
        base = cc * nsh
        lo = np.searchsorted(dst_s, base)
        hi = np.searchsorted(dst_s, base + nsh)
        cdst = dst_s[lo:hi] - base
        csrc = src_s[lo:hi]
        caed = a_edge_real[lo:hi]
        cw = cdst // P
        wstarts = np.searchsorted(cw, np.arange(nw))
        wends = np.searchsorted(cw, np.arange(nw) + 1)
        for w in range(nw):
            wsz = lw if w == nw - 1 else P
            s, e = wstarts[w], wends[w]
            self_nodes = base + w * P + np.arange(wsz)
            srcs = np.concatenate([csrc[s:e], self_nodes])
            dls = np.concatenate([cdst[s:e] - w * P,
                                  np.arange(wsz, dtype=np.int64)])
            aeds = np.concatenate([caed[s:e], a_edge_self[self_nodes]], axis=0)
            k = srcs.shape[0]
            assert k <= P * T
            pj = np.arange(k) % P
            tj = np.arange(k) // P
            src_arr[cc, w, pj, tj] = srcs
            dl_arr[cc, w, pj, tj] = dls
            aed_arr[cc, w, pj, tj] = aeds

    # ffn / ln weights
    w1 = np.asarray(inputs["ffn_w1"], np.float32)
    b1 = np.asarray(inputs["ffn_b1"], np.float32)
    w2 = np.asarray(inputs["ffn_w2"], np.float32)
    b2 = np.ascontiguousarray(np.asarray(inputs["ffn_b2"], np.float32))
    ln1g = np.ascontiguousarray(np.asarray(inputs["ln1_g"], np.float32))
    ln1b = np.ascontiguousarray(np.asarray(inputs["ln1_b"], np.float32))
    ln2g = np.ascontiguousarray(np.asarray(inputs["ln2_g"], np.float32))
    ln2b = np.ascontiguousarray(np.asarray(inputs["ln2_b"], np.float32))
    nmi = w1.shape[2] // P
    b1c = np.ascontiguousarray(
        b1.reshape(n_layers, nmi, P).transpose(0, 2, 1))

    # per-window tile counts: max over cores so the SPMD program is shared
    Tw = np.maximum(1, np.ceil(counts.max(axis=0) / P)).astype(int)  # [nw]

    # one-hot scatter matrices, bf16, layer-invariant, interleaved with
    # their transposes: ohh[cc, w, p, t, 0, :]=oh, [..., 1, :]=ohT
    eye = np.arange(P, dtype=np.int32)
    oh_all = (dl_arr[..., None] == eye).astype(NPBF)       # [cc,nw,P,T,P]
    ohT_all = oh_all.transpose(0, 1, 4, 3, 2)
    ohh_all = np.ascontiguousarray(
        np.stack([oh_all, ohT_all], axis=4))               # [cc,nw,P,T,2,P]
    del oh_all, ohT_all

    w1b = np.ascontiguousarray(w1.astype(NPBF))
    w2b = np.ascontiguousarray(w2.astype(NPBF))
    wextb = np.ascontiguousarray(W_ext.astype(NPBF))
    aedb = np.ascontiguousarray(aed_arr.astype(NPBF))

    in_maps = []
    for cc in range(n_cores):
        in_maps.append({
            "x0": np.ascontiguousarray(x[cc * nsh:(cc + 1) * nsh]),
            "w_ext": wextb,
            "gat_bias": gat_bias.copy(),
            "w1_d": w1b, "b1c_d": b1c, "w2_d": w2b, "b2_d": b2,
            "ln1g_d": ln1g, "ln1b_d": ln1b,
            "ln2g_d": ln2g, "ln2b_d": ln2b,
            "src_d": np.ascontiguousarray(src_arr[cc]),
            "aed_d": aedb[cc],
            "ohh_d": ohh_all[cc],
        })
    cfg = dict(n_nodes=n_nodes, nsh=nsh, nw=nw, lw=lw, T=T,
               n_layers=n_layers, n_cores=n_cores, d=d, h=h,
               dff=w1.shape[2], Tw=[int(t) for t in Tw])
    return in_maps, cfg


# ---------------------------------------------------------------------------
# device kernel
# ---------------------------------------------------------------------------

def _legalize_single_wait(nc):
    """This walrus build allows at most one sync wait per instruction.

    Split extra waits onto standalone EventSemaphore instructions right
    before the owner (same engine => identical semantics).
    """
    def fix(blocks):
        n = 0
        for blk in blocks:
            newl = []
            for inst in list(blk.instructions):
                si = getattr(inst, "sync_info", None)
                ow = list(si.on_wait) if (si is not None and si.on_wait) else []
                if len(ow) > 1:
                    for j, wt in enumerate(ow[:-1]):
                        newl.append(mybir.InstEventSemaphore(
                            name=f"{inst.name}wf{j}",
                            sync_info=mybir.SyncInfo(on_wait=[wt],
                                                     on_update=[]),
                            engine=inst.engine,
                        ))
                    inst.sync_info = mybir.SyncInfo(
                        on_wait=[ow[-1]], on_update=list(si.on_update))
                    n += 1
                newl.append(inst)
            blk.instructions = newl
            subs = list(blk.blocks) if getattr(blk, "blocks", None) else []
            if subs:
                n += fix(subs)
        return n

    for f in nc.m.functions:
        fix(list(f.blocks))


def build_nc(cfg, legalize=True):
    n_nodes = cfg["n_nodes"]
    nsh = cfg["nsh"]
    NW = cfg["nw"]
    LW = cfg["lw"]
    T = cfg["T"]
    LAYERS = cfg["n_layers"]
    n_cores = cfg["n_cores"]
    d = cfg["d"]
    h = cfg["h"]
    dff = cfg["dff"]
    KT = d // P          # K-tiles for D-contraction (2)
    NMI = dff // P       # M-tiles for dff (8)
    TBL = d + h          # 264: x_proj | a_src
    EXT = d + 2 * h      # 272: x_proj | a_src | a_dst

    abl = set((os.environ.get("GAT_ABL") or "").split(","))
    Tw = cfg.get("Tw") or [T] * NW

    nc = bass.Bass("TRN2", target_bir_lowering=False, debug=False,
                   num_devices=n_cores,
                   dynamic_dma_scratch_size=int(
                       os.environ.get("GAT_DMASCRATCH", "16384")))

    x0 = nc.dram_tensor("x0", [nsh, d], F32, kind="ExternalInput")
    w_ext = nc.dram_tensor("w_ext", [d, EXT], BF16, kind="ExternalInput")
    gat_bias = nc.dram_tensor("gat_bias", [d], F32, kind="ExternalInput")
    w1_d = nc.dram_tensor("w1_d", [LAYERS, d, dff], BF16,
                          kind="ExternalInput")
    b1c_d = nc.dram_tensor("b1c_d", [LAYERS, P, NMI], F32,
                           kind="ExternalInput")
    w2_d = nc.dram_tensor("w2_d", [LAYERS, dff, d], BF16,
                          kind="ExternalInput")
    b2_d = nc.dram_tensor("b2_d", [LAYERS, d], F32, kind="ExternalInput")
    ln1g_d = nc.dram_tensor("ln1g_d", [LAYERS, d], F32, kind="ExternalInput")
    ln1b_d = nc.dram_tensor("ln1b_d", [LAYERS, d], F32, kind="ExternalInput")
    ln2g_d = nc.dram_tensor("ln2g_d", [LAYERS, d], F32, kind="ExternalInput")
    ln2b_d = nc.dram_tensor("ln2b_d", [LAYERS, d], F32, kind="ExternalInput")
    src_d = nc.dram_tensor("src_d", [NW, P, T], I32, kind="ExternalInput")
    aed_d = nc.dram_tensor("aed_d", [NW, P, T, h], BF16,
                           kind="ExternalInput")
    ohh_d = nc.dram_tensor("ohh_d", [NW, P, T, 2, P], BF16,
                           kind="ExternalInput")
    out_d = nc.dram_tensor("out_d", [nsh, d], F32, kind="ExternalOutput")

    G = min(7, NW)  # windows per group (epilogue/FFN batching granularity)
    groups = [(g0, min(g0 + G, NW) - g0) for g0 in range(0, NW, G)]

    def wsz(w):
        return LW if w == NW - 1 else P

    with tile.TileContext(nc) as tc:
        with (
            tc.tile_pool(name="const", bufs=1) as const,
            tc.tile_pool(name="wpool", bufs=2) as wpool,
            tc.tile_pool(name="big", bufs=1) as big,
            tc.tile_pool(name="work", bufs=2) as work,
            tc.tile_pool(name="edge", bufs=3) as edge,
            tc.tile_pool(name="small", bufs=2) as small,
            tc.tile_pool(name="stats", bufs=1) as stats,
            tc.tile_pool(name="psum", bufs=1, space="PSUM") as psum,
            tc.tile_pool(name="dram", bufs=1, space="DRAM") as dram,
        ):
            # ---------- constants ----------
            if "reset" in (os.environ.get("GAT_OPT") or ""):
                nc.gpsimd.dma_reset()
            from concourse.masks import make_identity
            identf = const.tile([P, P], F32)
            make_identity(nc, identf[:, :])
            wext_sb = const.tile([P, KT, EXT], BF16)
            nc.sync.dma_start(
                out=wext_sb[:, :, :],
                in_=w_ext[:, :].rearrange("(kk p) c -> p kk c", p=P))
            bias_b = const.tile([P, d], F32)
            nc.sync.dma_start(out=bias_b[:, :],
                              in_=gat_bias[None, :].to_broadcast((P, d)))
            eps_t = const.tile([P, 1], F32)
            nc.vector.memset(eps_t[:, :], EPS)

            # ---------- persistent node state ----------
            m_sb = big.tile([P, NW, d], F32)
            if LW < P:
                nc.vector.memset(m_sb[:, NW - 1, :], 0.0)
            for w in range(NW):
                nc.sync.dma_start(out=m_sb[:wsz(w), w, :],
                                  in_=x0[w * P:w * P + wsz(w), :])
            adst_sb = big.tile([P, NW, h], BF16)

            REP = int(os.environ.get("GAT_REPEAT", "1"))
            for layer in [ly for _ in range(REP) for ly in range(LAYERS)]:
                # ---------- per-layer weights ----------
                w1_sb = wpool.tile([P, KT, dff], BF16, tag="w1")
                nc.sync.dma_start(
                    out=w1_sb[:, :, :],
                    in_=w1_d[layer, :, :].rearrange("(kk p) f -> p kk f", p=P))
                w2_sb = wpool.tile([P, NMI, d], BF16, tag="w2")
                nc.sync.dma_start(
                    out=w2_sb[:, :, :],
                    in_=w2_d[layer, :, :].rearrange("(kk p) f -> p kk f", p=P))
                b1c_sb = wpool.tile([P, NMI], F32, tag="b1c")
                nc.sync.dma_start(out=b1c_sb[:, :], in_=b1c_d[layer, :, :])
                b2_b = wpool.tile([P, d], F32, tag="b2")
                nc.sync.dma_start(
                    out=b2_b[:, :],
                    in_=b2_d[layer:layer + 1, :].to_broadcast((P, d)))
                ln1g_b = wpool.tile([P, d], F32, tag="ln1g")
                nc.sync.dma_start(
                    out=ln1g_b[:, :],
                    in_=ln1g_d[layer:layer + 1, :].to_broadcast((P, d)))
                ln1b_b = wpool.tile([P, d], F32, tag="ln1b")
                nc.sync.dma_start(
                    out=ln1b_b[:, :],
                    in_=ln1b_d[layer:layer + 1, :].to_broadcast((P, d)))
                ln2g_b = wpool.tile([P, d], F32, tag="ln2g")
                nc.sync.dma_start(
                    out=ln2g_b[:, :],
                    in_=ln2g_d[layer:layer + 1, :].to_broadcast((P, d)))
                ln2b_b = wpool.tile([P, d], F32, tag="ln2b")
                nc.sync.dma_start(
                    out=ln2b_b[:, :],
                    in_=ln2b_d[layer:layer + 1, :].to_broadcast((P, d)))

                shard_t = dram.tile([nsh, TBL], BF16, tag="shard")
                table_t = dram.tile([n_nodes, TBL], BF16, tag="table",
                                    addr_space="Shared")

                # ---------- phase A: x_proj | a_src | a_dst ----------
                for w in range(NW):
                    mT_sb = work.tile([P, KT, P], BF16, tag="mT")
                    for kk in range(KT):
                        tp = psum.tile([P, P], F32, tag="t1", bufs=2)
                        nc.tensor.transpose(
                            tp[:, :], m_sb[:, w, kk * P:(kk + 1) * P],
                            identf[:, :])
                        nc.scalar.activation(mT_sb[:, kk, :], tp[:, :],
                                             AF.Copy)
                    pj = psum.tile([P, EXT], F32, tag="t2", bufs=2)
                    for kk in range(KT):
                        nc.tensor.matmul(pj[:, :], lhsT=mT_sb[:, kk, :],
                                         rhs=wext_sb[:, kk, :],
                                         start=(kk == 0), stop=(kk == KT - 1))
                    prj = work.tile([P, EXT], BF16, tag="prj")
                    nc.scalar.activation(prj[:, :], pj[:, :], AF.Copy)
                    nc.sync.dma_start(out=shard_t[w * P:w * P + wsz(w), :],
                                      in_=prj[:wsz(w), :TBL])
                    nc.vector.tensor_copy(adst_sb[:, w, :], prj[:, TBL:EXT])

                # ---------- all-gather the projection table ----------
                if "noag" not in abl:
                    nc.gpsimd.collective_compute(
                        "AllGather",
                        AX.bypass,
                        replica_groups=[list(range(n_cores))],
                        ins=[shard_t.opt()],
                        outs=[table_t.opt()],
                    )

                # ---------- grouped phases B (edges) + C (FFN) ----------
                for g0, gn in groups:
                    gsl = slice(g0, g0 + gn)
                    g_grp = work.tile([P, G, TBL], F32, tag="ggrp")
                    # group metadata in one DMA each
                    idx_g = small.tile([P, G, T], I32, tag="idx")
                    nc.sync.dma_start(out=idx_g[:, :gn, :],
                                      in_=src_d[gsl, :, :].rearrange(
                                          "g p t -> p g t"))
                    aed_g = work.tile([P, G, T, h], BF16, tag="aed")
                    nc.sync.dma_start(out=aed_g[:, :gn, :, :],
                                      in_=aed_d[gsl, :, :, :].rearrange(
                                          "g p t hh -> p g t hh"))
                    for wi in range(gn):
                        w = g0 + wi
                        tw = Tw[w]
                        ohh_w = edge.tile([P, T, 2, P], BF16, tag="ohh")
                        nc.sync.dma_start(out=ohh_w[:, :tw, :, :],
                                          in_=ohh_d[w, :, :tw, :, :])
                        gat_w = edge.tile([P, T, TBL], BF16, tag="gat")
                        if "nogat" in abl:
                            nc.vector.memset(gat_w[:, 0, :], 0.5)
                        else:
                            for t in range(tw):
                                nc.gpsimd.indirect_dma_start(
                                    out=gat_w[:, t, :],
                                    out_offset=None,
                                    in_=table_t[:, :],
                                    in_offset=bass.IndirectOffsetOnAxis(
                                        ap=idx_g[:, wi, t:t + 1], axis=0),
                                )
                        # per-edge a_dst via tiny matmuls: [edge,h] per t
                        adxp = psum.tile([P, T * h], F32, tag="t3", bufs=2)
                        for t in range(tw):
                            nc.tensor.matmul(adxp[:, t * h:(t + 1) * h],
                                             lhsT=ohh_w[:, t, 1, :],
                                             rhs=adst_sb[:, w, :],
                                             start=True, stop=True)
                        adx_w = small.tile([P, T, h], BF16, tag="adx")
                        nc.scalar.activation(
                            adx_w[:, :tw, :],
                            adxp[:, :tw * h].rearrange(
                                "p (t hh) -> p t hh", hh=h),
                            AF.Copy)
                        # alpha = a_src[src]+a_dst[dst]+a_edge; lrelu; exp
                        alpha = small.tile([P, T, h], BF16, tag="alpha")
                        nc.vector.tensor_tensor(out=alpha[:, :tw, :],
                                                in0=aed_g[:, wi, :tw, :],
                                                in1=gat_w[:, :tw, d:TBL],
                                                op=AX.add)
                        nc.vector.tensor_tensor(out=alpha[:, :tw, :],
                                                in0=alpha[:, :tw, :],
                                                in1=adx_w[:, :tw, :],
                                                op=AX.add)
                        nc.vector.tensor_scalar(out=adx_w[:, :tw, :],
                                                in0=alpha[:, :tw, :],
                                                scalar1=NEG_SLOPE,
                                                scalar2=None, op0=AX.mult)
                        nc.vector.tensor_tensor(out=alpha[:, :tw, :],
                                                in0=alpha[:, :tw, :],
                                                in1=adx_w[:, :tw, :],
                                                op=AX.max)
                        nc.scalar.activation(gat_w[:, :tw, d:TBL],
                                             alpha[:, :tw, :], AF.Exp)
                        # messages: x_proj *= ex (broadcast over channels)
                        nc.vector.tensor_tensor(
                            out=gat_w[:, :tw, 0:d].rearrange(
                                "p t (hh c) -> p t hh c", c=C),
                            in0=gat_w[:, :tw, 0:d].rearrange(
                                "p t (hh c) -> p t hh c", c=C),
                            in1=gat_w[:, :tw, d:TBL][:, :, :,
                                                     None].to_broadcast(
                                (P, tw, h, C)),
                            op=AX.mult)
                        # accumulate  [window, x_sum | ex_sum]
                        acc = psum.tile([P, TBL], F32, tag="t2", bufs=2)
                        nts = 1 if "noacc" in abl else tw
                        for t in range(nts):
                            nc.tensor.matmul(acc[:, :], lhsT=ohh_w[:, t, 0, :],
                                             rhs=gat_w[:, t, :],
                                             start=(t == 0),
                                             stop=(t == nts - 1))
                        nc.scalar.activation(g_grp[:, wi, :], acc[:, :],
                                             AF.Copy)

                    # ---- group epilogue: softmax div + bias + resid + LN1
                    den = g_grp[:, :gn, d:TBL]
                    nc.vector.tensor_scalar(out=den, in0=den, scalar1=1e-30,
                                            scalar2=None, op0=AX.max)
                    rec = stats.tile([P, G, h], F32, tag="rec", bufs=2)
                    nc.vector.reciprocal(rec[:, :gn, :], den)
                    nc.vector.tensor_tensor(
                        out=g_grp[:, :gn, 0:d].rearrange(
                            "p w (hh c) -> p w hh c", c=C),
                        in0=g_grp[:, :gn, 0:d].rearrange(
                            "p w (hh c) -> p w hh c", c=C),
                        in1=rec[:, :gn, :, None].to_broadcast((P, gn, h, C)),
                        op=AX.mult)
                    nc.vector.tensor_tensor(
                        out=g_grp[:, :gn, 0:d], in0=g_grp[:, :gn, 0:d],
                        in1=bias_b[:, None, :].to_broadcast((P, gn, d)),
                        op=AX.add)
                    nc.vector.tensor_tensor(out=m_sb[:, gsl, :],
                                            in0=m_sb[:, gsl, :],
                                            in1=g_grp[:, :gn, 0:d],
                                            op=AX.add)
                    _layernorm_batched(nc, stats, m_sb[:, gsl, :],
                                       g_grp[:, :gn, 0:d], ln1g_b, ln1b_b,
                                       eps_t, gn, d)

                    # ---- FFN for the group's windows
                    for wi in range(gn):
                        w = g0 + wi
                        mT_sb = work.tile([P, KT, P], BF16, tag="mT")
                        for kk in range(KT):
                            tp = psum.tile([P, P], F32, tag="t1", bufs=2)
                            nc.tensor.transpose(
                                tp[:, :], m_sb[:, w, kk * P:(kk + 1) * P],
                                identf[:, :])
                            nc.scalar.activation(mT_sb[:, kk, :], tp[:, :],
                                                 AF.Copy)
                        h1T = work.tile([P, NMI, P], BF16, tag="h1T")
                        half = NMI // 2
                        for hh2 in range(2):
                            tag = "t2" if hh2 == 0 else "t3"
                            hp = psum.tile([P, half, P], F32, tag=tag, bufs=2)
                            for sl in range(half):
                                mi = hh2 * half + sl
                                for kk in range(KT):
                                    nc.tensor.matmul(
                                        hp[:, sl, :],
                                        lhsT=w1_sb[:, kk, mi * P:(mi + 1) * P],
                                        rhs=mT_sb[:, kk, :],
                                        start=(kk == 0), stop=(kk == KT - 1))
                            nc.vector.tensor_tensor(
                                out=h1T[:, hh2 * half:(hh2 + 1) * half, :],
                                in0=hp[:, :, :],
                                in1=b1c_sb[:, hh2 * half:(hh2 + 1) * half,
                                           None].to_broadcast((P, half, P)),
                                op=AX.add)
                        nc.scalar.activation(h1T[:, :, :], h1T[:, :, :],
                                             AF.Relu)
                        h2p = psum.tile([P, d], F32, tag="t1", bufs=2)
                        for mi in range(NMI):
                            nc.tensor.matmul(h2p[:, :], lhsT=h1T[:, mi, :],
                                             rhs=w2_sb[:, mi, :],
                                             start=(mi == 0),
                                             stop=(mi == NMI - 1))
                        nc.scalar.activation(g_grp[:, wi, 0:d], h2p[:, :],
                                             AF.Copy)

                    # ---- h + b2 + resid + LN2
                    nc.vector.tensor_tensor(
                        out=g_grp[:, :gn, 0:d], in0=g_grp[:, :gn, 0:d],
                        in1=b2_b[:, None, :].to_broadcast((P, gn, d)),
                        op=AX.add)
                    nc.vector.tensor_tensor(out=m_sb[:, gsl, :],
                                            in0=m_sb[:, gsl, :],
                                            in1=g_grp[:, :gn, 0:d],
                                            op=AX.add)
                    _layernorm_batched(nc, stats, m_sb[:, gsl, :],
                                       g_grp[:, :gn, 0:d], ln2g_b, ln2b_b,
                                       eps_t, gn, d)

            # ---------- output ----------
            for w in range(NW):
                nc.sync.dma_start(out=out_d[w * P:w * P + wsz(w), :],
                                  in_=m_sb[:wsz(w), w, :])
    if legalize:
        _legalize_single_wait(nc)
    return nc


def _layernorm_batched(nc, stats, m_ap, scratch_ap, g_b, b_b, eps_t, nwin, d):
    """In-place LayerNorm over the feature axis of m_ap [P, nwin, d]."""
    ssum = stats.tile([P, nwin], F32, tag="ssum", bufs=2)
    nc.vector.tensor_reduce(out=ssum[:, :], in_=m_ap,
                            axis=mybir.AxisListType.X, op=AX.add)
    nc.vector.tensor_tensor(out=scratch_ap, in0=m_ap, in1=m_ap, op=AX.mult)
    ssq = stats.tile([P, nwin], F32, tag="ssq", bufs=2)
    nc.vector.tensor_reduce(out=ssq[:, :], in_=scratch_ap,
                            axis=mybir.AxisListType.X, op=AX.add)
    mu = stats.tile([P, nwin], F32, tag="mu", bufs=2)
    nc.vector.tensor_scalar(out=mu[:, :], in0=ssum[:, :], scalar1=1.0 / d,
                            scalar2=None, op0=AX.mult)
    var = stats.tile([P, nwin], F32, tag="var", bufs=2)
    nc.vector.tensor_scalar(out=var[:, :], in0=ssq[:, :], scalar1=1.0 / d,
                            scalar2=None, op0=AX.mult)
    mu2 = stats.tile([P, nwin], F32, tag="mu2", bufs=2)
    nc.vector.tensor_tensor(out=mu2[:, :], in0=mu[:, :], in1=mu[:, :],
                            op=AX.mult)
    nc.vector.tensor_tensor(out=var[:, :], in0=var[:, :], in1=mu2[:, :],
                            op=AX.subtract)
    nc.scalar.activation(var[:, :], var[:, :], AF.Sqrt, bias=eps_t[:, :1])
    rstd = stats.tile([P, nwin], F32, tag="rstd", bufs=2)
    nc.vector.reciprocal(rstd[:, :], var[:, :])
    nc.vector.tensor_tensor(out=m_ap, in0=m_ap,
                            in1=mu[:, :, None].to_broadcast((P, nwin, d)),
                            op=AX.subtract)
    nc.vector.tensor_tensor(out=m_ap, in0=m_ap,
                            in1=rstd[:, :, None].to_broadcast((P, nwin, d)),
                            op=AX.mult)
    nc.vector.tensor_tensor(out=m_ap, in0=m_ap,
                            in1=g_b[:, None, :].to_broadcast((P, nwin, d)),
                            op=AX.mult)
    nc.vector.tensor_tensor(out=m_ap, in0=m_ap,
                            in1=b_b[:, None, :].to_broadcast((P, nwin, d)),
                            op=AX.add)


# ---------------------------------------------------------------------------
# entry point
# ---------------------------------------------------------------------------

_LAST_RESULTS = {}


def kernel(**inputs):
    n_nodes = inputs["x"].shape[0]
    n_edges = inputs["edge_index"].shape[1]
    n_layers = inputs["ffn_w1"].shape[0]
    in_maps, cfg = _prep(inputs, n_nodes, n_edges, n_layers, NCORES)
    nc = build_nc(cfg)
    res = run_bass_kernel_spmd(
        nc, in_maps, list(range(NCORES)),
        trace=bool(int(os.environ.get("GAT_TRACE", "0"))),
    )
    _LAST_RESULTS["res"] = res
    out = np.concatenate([res.results[cc]["out_d"] for cc in range(NCORES)],
                         axis=0)
    return out


# revision 7
# speedup vs baseline: 1.0424x; 1.0106x over previous
"""GAT message-passing kernel for Trainium2, 8 NeuronCores.

Design (vs the straightforward implementation):
  - One-hot scatter matrices and their transposes are built on the host
    (layer-invariant), stored interleaved bf16 in DRAM (one tensor, one
    DMA per window).  Building them on-device with DVE is_equal
    broadcasts cost ~40ms of the original 97ms.
  - Per-edge a_dst comes from tiny PE matmuls (lhsT=ohT, rhs=adst window
    column) instead of thousands of small indirect DMAs.
  - The all-gathered projection table [x_proj | a_src] is bf16: halves
    the AllGather and per-edge gather traffic.  All big matmuls take
    bf16 inputs (1 cycle/row on PE vs 4 for fp32); PSUM accumulation,
    the residual stream, LayerNorms and the softmax division stay fp32.
  - Per-window gather tile counts are trimmed to the max over cores
    (SPMD-shared program) instead of the global max.
  - Edge-phase tiles are quad-buffered and the aed+a_dst add is issued
    before the gathers complete to shorten the post-gather chain.
"""

import math
import os

import numpy as np
import ml_dtypes

import concourse.bass as bass
import concourse.mybir as mybir
import concourse.tile as tile
from concourse.bass_utils import run_bass_kernel_spmd

# problem dims (hardcoded per contract)
N, E, D, H, ED, L = 50000, 800000, 256, 8, 16, 6
C = D // H  # 32
DFF = 4 * D  # 1024
NEG_SLOPE = 0.2
EPS = 1e-5
NCORES = 8
P = 128

F32 = mybir.dt.float32
BF16 = mybir.dt.bfloat16
I32 = mybir.dt.int32
AX = mybir.AluOpType
AF = mybir.ActivationFunctionType

NEG_BIG = -1.0e30  # padded-edge a_edge => exp -> 0
NPBF = ml_dtypes.bfloat16


# ---------------------------------------------------------------------------
# host-side preprocessing
# ---------------------------------------------------------------------------

def _prep(inputs, n_nodes, n_edges, n_layers, n_cores):
    """Builds per-core input maps + the compile-time config."""
    x = np.asarray(inputs["x"], np.float32)
    edge_index = np.asarray(inputs["edge_index"])
    edge_attr = np.asarray(inputs["edge_attr"], np.float32)
    W = np.asarray(inputs["W"], np.float32)
    att_src = np.asarray(inputs["att_src"], np.float32)
    att_dst = np.asarray(inputs["att_dst"], np.float32)
    att_edge = np.asarray(inputs["att_edge"], np.float32)
    W_edge = np.asarray(inputs["W_edge"], np.float32)
    gat_bias = np.asarray(inputs["bias"], np.float32)

    d = W.shape[0]
    h = att_src.shape[0]
    c = att_src.shape[1]

    nsh = n_nodes // n_cores
    assert nsh * n_cores == n_nodes
    nw = math.ceil(nsh / P)
    lw = nsh - (nw - 1) * P  # rows in last window

    # fold attention vectors into the projection:  m @ W_ext ->
    # [x_proj | a_src | a_dst]
    S_src = np.zeros((d, h), np.float32)
    S_dst = np.zeros((d, h), np.float32)
    for hh in range(h):
        S_src[hh * c:(hh + 1) * c, hh] = att_src[hh]
        S_dst[hh * c:(hh + 1) * c, hh] = att_dst[hh]
    W_ext = np.concatenate([W, W @ S_src, W @ S_dst], axis=1)  # [d, d+2h]

    # per-edge a_edge = sum_c (ea @ W_edge)[h,c] * att_edge[h,c] = ea @ V
    V = np.zeros((W_edge.shape[0], h), np.float32)
    for hh in range(h):
        V[:, hh] = W_edge[:, hh * c:(hh + 1) * c] @ att_edge[hh]
    src0 = edge_index[0].astype(np.int64)
    dst0 = edge_index[1].astype(np.int64)

    # self-loop edge_attr = mean of incoming real-edge attrs (PyG default)
    order = np.argsort(dst0, kind="stable")
    dst_s = dst0[order]
    src_s = src0[order]
    ea_sum = np.zeros((n_nodes, W_edge.shape[0]), np.float32)
    if n_edges > 0:
        starts = np.flatnonzero(np.r_[True, dst_s[1:] != dst_s[:-1]])
        sums = np.add.reduceat(edge_attr[order], starts, axis=0)
        ea_sum[dst_s[starts]] = sums
    deg = np.bincount(dst0, minlength=n_nodes).astype(np.float32)
    ea_mean = ea_sum / np.maximum(deg, 1.0)[:, None]

    a_edge_real = (edge_attr @ V).astype(np.float32)[order]  # dst-sorted
    a_edge_self = (ea_mean @ V).astype(np.float32)

    # build per-core padded edge structure
    counts = np.zeros((n_cores, nw), np.int64)
    core_all = dst_s // nsh
    lw_all = (dst_s - core_all * nsh) // P
    np.add.at(counts, (core_all, lw_all), 1)
    node_ids = np.arange(n_nodes, dtype=np.int64)
    sc = node_ids // nsh
    slw = (node_ids - sc * nsh) // P
    np.add.at(counts, (sc, slw), 1)
    T = int(math.ceil(counts.max() / P))

    src_arr = np.zeros((n_cores, nw, P, T), np.int32)
    dl_arr = np.zeros((n_cores, nw, P, T), np.int32)
    aed_arr = np.full((n_cores, nw, P, T, h), NEG_BIG, np.float32)

    for cc in range(n_cores):
        base = cc * nsh
        lo = np.searchsorted(dst_s, base)
        hi = np.searchsorted(dst_s, base + nsh)
        cdst = dst_s[lo:hi] - base
        csrc = src_s[lo:hi]
        caed = a_edge_real[lo:hi]
        cw = cdst // P
        wstarts = np.searchsorted(cw, np.arange(nw))
        wends = np.searchsorted(cw, np.arange(nw) + 1)
        for w in range(nw):
            wsz = lw if w == nw - 1 else P
            s, e = wstarts[w], wends[w]
            self_nodes = base + w * P + np.arange(wsz)
            srcs = np.concatenate([csrc[s:e], self_nodes])
            dls = np.concatenate([cdst[s:e] - w * P,
                                  np.arange(wsz, dtype=np.int64)])
            aeds = np.concatenate([caed[s:e], a_edge_self[self_nodes]], axis=0)
            k = srcs.shape[0]
            assert k <= P * T
            pj = np.arange(k) % P
            tj = np.arange(k) // P
            src_arr[cc, w, pj, tj] = srcs
            dl_arr[cc, w, pj, tj] = dls
            aed_arr[cc, w, pj, tj] = aeds

    # ffn / ln weights
    w1 = np.asarray(inputs["ffn_w1"], np.float32)
    b1 = np.asarray(inputs["ffn_b1"], np.float32)
    w2 = np.asarray(inputs["ffn_w2"], np.float32)
    b2 = np.ascontiguousarray(np.asarray(inputs["ffn_b2"], np.float32))
    ln1g = np.ascontiguousarray(np.asarray(inputs["ln1_g"], np.float32))
    ln1b = np.ascontiguousarray(np.asarray(inputs["ln1_b"], np.float32))
    ln2g = np.ascontiguousarray(np.asarray(inputs["ln2_g"], np.float32))
    ln2b = np.ascontiguousarray(np.asarray(inputs["ln2_b"], np.float32))
    nmi = w1.shape[2] // P
    b1c = np.ascontiguousarray(
        b1.reshape(n_layers, nmi, P).transpose(0, 2, 1))

    # per-window tile counts: max over cores so the SPMD program is shared
    Tw = np.maximum(1, np.ceil(counts.max(axis=0) / P)).astype(int)  # [nw]

    # one-hot scatter matrices, bf16, layer-invariant, interleaved with
    # their transposes: ohh[cc, w, p, t, 0, :]=oh, [..., 1, :]=ohT
    eye = np.arange(P, dtype=np.int32)
    oh_all = (dl_arr[..., None] == eye).astype(NPBF)       # [cc,nw,P,T,P]
    ohT_all = oh_all.transpose(0, 1, 4, 3, 2)
    ohh_all = np.ascontiguousarray(
        np.stack([oh_all, ohT_all], axis=4))               # [cc,nw,P,T,2,P]
    del oh_all, ohT_all

    w1b = np.ascontiguousarray(w1.astype(NPBF))
    w2b = np.ascontiguousarray(w2.astype(NPBF))
    wextb = np.ascontiguousarray(W_ext.astype(NPBF))
    aedb = np.ascontiguousarray(aed_arr.astype(NPBF))

    in_maps = []
    for cc in range(n_cores):
        in_maps.append({
            "x0": np.ascontiguousarray(x[cc * nsh:(cc + 1) * nsh]),
            "w_ext": wextb,
            "gat_bias": gat_bias.copy(),
            "w1_d": w1b, "b1c_d": b1c, "w2_d": w2b, "b2_d": b2,
            "ln1g_d": ln1g, "ln1b_d": ln1b,
            "ln2g_d": ln2g, "ln2b_d": ln2b,
            "src_d": np.ascontiguousarray(src_arr[cc]),
            "aed_d": aedb[cc],
            "ohh_d": ohh_all[cc],
        })
    cfg = dict(n_nodes=n_nodes, nsh=nsh, nw=nw, lw=lw, T=T,
               n_layers=n_layers, n_cores=n_cores, d=d, h=h,
               dff=w1.shape[2], Tw=[int(t) for t in Tw])
    return in_maps, cfg


# ---------------------------------------------------------------------------
# device kernel
# ---------------------------------------------------------------------------

def _legalize_single_wait(nc):
    """This walrus build allows at most one sync wait per instruction.

    Split extra waits onto standalone EventSemaphore instructions right
    before the owner (same engine => identical semantics).
    """
    def fix(blocks):
        n = 0
        for blk in blocks:
            newl = []
            for inst in list(blk.instructions):
                si = getattr(inst, "sync_info", None)
                ow = list(si.on_wait) if (si is not None and si.on_wait) else []
                if len(ow) > 1:
                    for j, wt in enumerate(ow[:-1]):
                        newl.append(mybir.InstEventSemaphore(
                            name=f"{inst.name}wf{j}",
                            sync_info=mybir.SyncInfo(on_wait=[wt],
                                                     on_update=[]),
                            engine=inst.engine,
                        ))
                    inst.sync_info = mybir.SyncInfo(
                        on_wait=[ow[-1]], on_update=list(si.on_update))
                    n += 1
                newl.append(inst)
            blk.instructions = newl
            subs = list(blk.blocks) if getattr(blk, "blocks", None) else []
            if subs:
                n += fix(subs)
        return n

    for f in nc.m.functions:
        fix(list(f.blocks))


def build_nc(cfg, legalize=True):
    n_nodes = cfg["n_nodes"]
    nsh = cfg["nsh"]
    NW = cfg["nw"]
    LW = cfg["lw"]
    T = cfg["T"]
    LAYERS = cfg["n_layers"]
    n_cores = cfg["n_cores"]
    d = cfg["d"]
    h = cfg["h"]
    dff = cfg["dff"]
    KT = d // P          # K-tiles for D-contraction (2)
    NMI = dff // P       # M-tiles for dff (8)
    TBL = d + h          # 264: x_proj | a_src
    EXT = d + 2 * h      # 272: x_proj | a_src | a_dst

    abl = set((os.environ.get("GAT_ABL") or "").split(","))
    Tw = cfg.get("Tw") or [T] * NW

    nc = bass.Bass("TRN2", target_bir_lowering=False, debug=False,
                   num_devices=n_cores,
                   dynamic_dma_scratch_size=int(
                       os.environ.get("GAT_DMASCRATCH", "16384")))

    x0 = nc.dram_tensor("x0", [nsh, d], F32, kind="ExternalInput")
    w_ext = nc.dram_tensor("w_ext", [d, EXT], BF16, kind="ExternalInput")
    gat_bias = nc.dram_tensor("gat_bias", [d], F32, kind="ExternalInput")
    w1_d = nc.dram_tensor("w1_d", [LAYERS, d, dff], BF16,
                          kind="ExternalInput")
    b1c_d = nc.dram_tensor("b1c_d", [LAYERS, P, NMI], F32,
                           kind="ExternalInput")
    w2_d = nc.dram_tensor("w2_d", [LAYERS, dff, d], BF16,
                          kind="ExternalInput")
    b2_d = nc.dram_tensor("b2_d", [LAYERS, d], F32, kind="ExternalInput")
    ln1g_d = nc.dram_tensor("ln1g_d", [LAYERS, d], F32, kind="ExternalInput")
    ln1b_d = nc.dram_tensor("ln1b_d", [LAYERS, d], F32, kind="ExternalInput")
    ln2g_d = nc.dram_tensor("ln2g_d", [LAYERS, d], F32, kind="ExternalInput")
    ln2b_d = nc.dram_tensor("ln2b_d", [LAYERS, d], F32, kind="ExternalInput")
    src_d = nc.dram_tensor("src_d", [NW, P, T], I32, kind="ExternalInput")
    aed_d = nc.dram_tensor("aed_d", [NW, P, T, h], BF16,
                           kind="ExternalInput")
    ohh_d = nc.dram_tensor("ohh_d", [NW, P, T, 2, P], BF16,
                           kind="ExternalInput")
    out_d = nc.dram_tensor("out_d", [nsh, d], F32, kind="ExternalOutput")

    G = min(7, NW)  # windows per group (epilogue/FFN batching granularity)
    groups = [(g0, min(g0 + G, NW) - g0) for g0 in range(0, NW, G)]

    def wsz(w):
        return LW if w == NW - 1 else P

    with tile.TileContext(nc) as tc:
        with (
            tc.tile_pool(name="const", bufs=1) as const,
            tc.tile_pool(name="wpool", bufs=2) as wpool,
            tc.tile_pool(name="big", bufs=1) as big,
            tc.tile_pool(name="work", bufs=2) as work,
            tc.tile_pool(name="edge", bufs=4) as edge,
            tc.tile_pool(name="small", bufs=2) as small,
            tc.tile_pool(name="stats", bufs=1) as stats,
            tc.tile_pool(name="psum", bufs=1, space="PSUM") as psum,
            tc.tile_pool(name="dram", bufs=1, space="DRAM") as dram,
        ):
            # ---------- constants ----------
            if "reset" in (os.environ.get("GAT_OPT") or ""):
                nc.gpsimd.dma_reset()
            from concourse.masks import make_identity
            identf = const.tile([P, P], F32)
            make_identity(nc, identf[:, :])
            wext_sb = const.tile([P, KT, EXT], BF16)
            nc.sync.dma_start(
                out=wext_sb[:, :, :],
                in_=w_ext[:, :].rearrange("(kk p) c -> p kk c", p=P))
            bias_b = const.tile([P, d], F32)
            nc.sync.dma_start(out=bias_b[:, :],
                              in_=gat_bias[None, :].to_broadcast((P, d)))
            eps_t = const.tile([P, 1], F32)
            nc.vector.memset(eps_t[:, :], EPS)

            # ---------- persistent node state ----------
            m_sb = big.tile([P, NW, d], F32)
            if LW < P:
                nc.vector.memset(m_sb[:, NW - 1, :], 0.0)
            for w in range(NW):
                nc.sync.dma_start(out=m_sb[:wsz(w), w, :],
                                  in_=x0[w * P:w * P + wsz(w), :])
            adst_sb = big.tile([P, NW, h], BF16)

            REP = int(os.environ.get("GAT_REPEAT", "1"))
            for layer in [ly for _ in range(REP) for ly in range(LAYERS)]:
                # ---------- per-layer weights ----------
                w1_sb = wpool.tile([P, KT, dff], BF16, tag="w1")
                nc.sync.dma_start(
                    out=w1_sb[:, :, :],
                    in_=w1_d[layer, :, :].rearrange("(kk p) f -> p kk f", p=P))
                w2_sb = wpool.tile([P, NMI, d], BF16, tag="w2")
                nc.sync.dma_start(
                    out=w2_sb[:, :, :],
                    in_=w2_d[layer, :, :].rearrange("(kk p) f -> p kk f", p=P))
                b1c_sb = wpool.tile([P, NMI], F32, tag="b1c")
                nc.sync.dma_start(out=b1c_sb[:, :], in_=b1c_d[layer, :, :])
                b2_b = wpool.tile([P, d], F32, tag="b2")
                nc.sync.dma_start(
                    out=b2_b[:, :],
                    in_=b2_d[layer:layer + 1, :].to_broadcast((P, d)))
                ln1g_b = wpool.tile([P, d], F32, tag="ln1g")
                nc.sync.dma_start(
                    out=ln1g_b[:, :],
                    in_=ln1g_d[layer:layer + 1, :].to_broadcast((P, d)))
                ln1b_b = wpool.tile([P, d], F32, tag="ln1b")
                nc.sync.dma_start(
                    out=ln1b_b[:, :],
                    in_=ln1b_d[layer:layer + 1, :].to_broadcast((P, d)))
                ln2g_b = wpool.tile([P, d], F32, tag="ln2g")
                nc.sync.dma_start(
                    out=ln2g_b[:, :],
                    in_=ln2g_d[layer:layer + 1, :].to_broadcast((P, d)))
                ln2b_b = wpool.tile([P, d], F32, tag="ln2b")
                nc.sync.dma_start(
                    out=ln2b_b[:, :],
                    in_=ln2b_d[layer:layer + 1, :].to_broadcast((P, d)))

                shard_t = dram.tile([nsh, TBL], BF16, tag="shard")
                table_t = dram.tile([n_nodes, TBL], BF16, tag="table",
                                    addr_space="Shared")

                # ---------- phase A: x_proj | a_src | a_dst ----------
                for w in range(NW):
                    mT_sb = work.tile([P, KT, P], BF16, tag="mT")
                    for kk in range(KT):
                        tp = psum.tile([P, P], F32, tag="t1", bufs=2)
                        nc.tensor.transpose(
                            tp[:, :], m_sb[:, w, kk * P:(kk + 1) * P],
                            identf[:, :])
                        nc.scalar.activation(mT_sb[:, kk, :], tp[:, :],
                                             AF.Copy)
                    pj = psum.tile([P, EXT], F32, tag="t2", bufs=3)
                    for kk in range(KT):
                        nc.tensor.matmul(pj[:, :], lhsT=mT_sb[:, kk, :],
                                         rhs=wext_sb[:, kk, :],
                                         start=(kk == 0), stop=(kk == KT - 1))
                    prj = work.tile([P, EXT], BF16, tag="prj")
                    nc.scalar.activation(prj[:, :], pj[:, :], AF.Copy)
                    nc.sync.dma_start(out=shard_t[w * P:w * P + wsz(w), :],
                                      in_=prj[:wsz(w), :TBL])
                    nc.vector.tensor_copy(adst_sb[:, w, :], prj[:, TBL:EXT])

                # ---------- all-gather the projection table ----------
                if "noag" not in abl:
                    nc.gpsimd.collective_compute(
                        "AllGather",
                        AX.bypass,
                        replica_groups=[list(range(n_cores))],
                        ins=[shard_t.opt()],
                        outs=[table_t.opt()],
                    )

                # ---------- grouped phases B (edges) + C (FFN) ----------
                for g0, gn in groups:
                    gsl = slice(g0, g0 + gn)
                    g_grp = work.tile([P, G, TBL], F32, tag="ggrp")
                    # group metadata in one DMA each
                    idx_g = small.tile([P, G, T], I32, tag="idx")
                    nc.sync.dma_start(out=idx_g[:, :gn, :],
                                      in_=src_d[gsl, :, :].rearrange(
                                          "g p t -> p g t"))
                    aed_g = work.tile([P, G, T, h], BF16, tag="aed")
                    nc.sync.dma_start(out=aed_g[:, :gn, :, :],
                                      in_=aed_d[gsl, :, :, :].rearrange(
                                          "g p t hh -> p g t hh"))
                    for wi in range(gn):
                        w = g0 + wi
                        tw = Tw[w]
                        ohh_w = edge.tile([P, T, 2, P], BF16, tag="ohh")
                        nc.sync.dma_start(out=ohh_w[:, :tw, :, :],
                                          in_=ohh_d[w, :, :tw, :, :])
                        gat_w = edge.tile([P, T, TBL], BF16, tag="gat")
                        if "nogat" in abl:
                            nc.vector.memset(gat_w[:, 0, :], 0.5)
                        else:
                            for t in range(tw):
                                nc.gpsimd.indirect_dma_start(
                                    out=gat_w[:, t, :],
                                    out_offset=None,
                                    in_=table_t[:, :],
                                    in_offset=bass.IndirectOffsetOnAxis(
                                        ap=idx_g[:, wi, t:t + 1], axis=0),
                                )
                        # per-edge a_dst via tiny matmuls: [edge,h] per t
                        adxp = psum.tile([P, T * h], F32, tag="t3", bufs=2)
                        for t in range(tw):
                            nc.tensor.matmul(adxp[:, t * h:(t + 1) * h],
                                             lhsT=ohh_w[:, t, 1, :],
                                             rhs=adst_sb[:, w, :],
                                             start=True, stop=True)
                        adx_w = small.tile([P, T, h], BF16, tag="adx")
                        nc.scalar.activation(
                            adx_w[:, :tw, :],
                            adxp[:, :tw * h].rearrange(
                                "p (t hh) -> p t hh", hh=h),
                            AF.Copy)
                        # alpha = a_src[src]+a_dst[dst]+a_edge; lrelu; exp
                        # (aed + a_dst first: no gather dependency, so it
                        # overlaps with the indirect gathers)
                        alpha = small.tile([P, T, h], BF16, tag="alpha")
                        nc.vector.tensor_tensor(out=alpha[:, :tw, :],
                                                in0=aed_g[:, wi, :tw, :],
                                                in1=adx_w[:, :tw, :],
                                                op=AX.add)
                        nc.vector.tensor_tensor(out=alpha[:, :tw, :],
                                                in0=alpha[:, :tw, :],
                                                in1=gat_w[:, :tw, d:TBL],
                                                op=AX.add)
                        nc.vector.tensor_scalar(out=adx_w[:, :tw, :],
                                                in0=alpha[:, :tw, :],
                                                scalar1=NEG_SLOPE,
                                                scalar2=None, op0=AX.mult)
                        nc.vector.tensor_tensor(out=alpha[:, :tw, :],
                                                in0=alpha[:, :tw, :],
                                                in1=adx_w[:, :tw, :],
                                                op=AX.max)
                        nc.scalar.activation(gat_w[:, :tw, d:TBL],
                                             alpha[:, :tw, :], AF.Exp)
                        # messages: x_proj *= ex (broadcast over channels)
                        nc.vector.tensor_tensor(
                            out=gat_w[:, :tw, 0:d].rearrange(
                                "p t (hh c) -> p t hh c", c=C),
                            in0=gat_w[:, :tw, 0:d].rearrange(
                                "p t (hh c) -> p t hh c", c=C),
                            in1=gat_w[:, :tw, d:TBL][:, :, :,
                                                     None].to_broadcast(
                                (P, tw, h, C)),
                            op=AX.mult)
                        # accumulate  [window, x_sum | ex_sum]
                        acc = psum.tile([P, TBL], F32, tag="t2", bufs=3)
                        nts = 1 if "noacc" in abl else tw
                        for t in range(nts):
                            nc.tensor.matmul(acc[:, :], lhsT=ohh_w[:, t, 0, :],
                                             rhs=gat_w[:, t, :],
                                             start=(t == 0),
                                             stop=(t == nts - 1))
                        nc.scalar.activation(g_grp[:, wi, :], acc[:, :],
                                             AF.Copy)

                    # ---- group epilogue: softmax div + bias + resid + LN1
                    den = g_grp[:, :gn, d:TBL]
                    nc.vector.tensor_scalar(out=den, in0=den, scalar1=1e-30,
                                            scalar2=None, op0=AX.max)
                    rec = stats.tile([P, G, h], F32, tag="rec", bufs=2)
                    nc.vector.reciprocal(rec[:, :gn, :], den)
                    nc.vector.tensor_tensor(
                        out=g_grp[:, :gn, 0:d].rearrange(
                            "p w (hh c) -> p w hh c", c=C),
                        in0=g_grp[:, :gn, 0:d].rearrange(
                            "p w (hh c) -> p w hh c", c=C),
                        in1=rec[:, :gn, :, None].to_broadcast((P, gn, h, C)),
                        op=AX.mult)
                    nc.vector.tensor_tensor(
                        out=g_grp[:, :gn, 0:d], in0=g_grp[:, :gn, 0:d],
                        in1=bias_b[:, None, :].to_broadcast((P, gn, d)),
                        op=AX.add)
                    nc.vector.tensor_tensor(out=m_sb[:, gsl, :],
                                            in0=m_sb[:, gsl, :],
                                            in1=g_grp[:, :gn, 0:d],
                                            op=AX.add)
                    _layernorm_batched(nc, stats, m_sb[:, gsl, :],
                                       g_grp[:, :gn, 0:d], ln1g_b, ln1b_b,
                                       eps_t, gn, d)

                    # ---- FFN for the group's windows
                    for wi in range(gn):
                        w = g0 + wi
                        mT_sb = work.tile([P, KT, P], BF16, tag="mT")
                        for kk in range(KT):
                            tp = psum.tile([P, P], F32, tag="t1", bufs=2)
                            nc.tensor.transpose(
                                tp[:, :], m_sb[:, w, kk * P:(kk + 1) * P],
                                identf[:, :])
                            nc.scalar.activation(mT_sb[:, kk, :], tp[:, :],
                                                 AF.Copy)
                        h1T = work.tile([P, NMI, P], BF16, tag="h1T")
                        half = NMI // 2
                        for hh2 in range(2):
                            tag = "t2" if hh2 == 0 else "t3"
                            hp = psum.tile([P, half, P], F32, tag=tag,
                                           bufs=3 if hh2 == 0 else 2)
                            for sl in range(half):
                                mi = hh2 * half + sl
                                for kk in range(KT):
                                    nc.tensor.matmul(
                                        hp[:, sl, :],
                                        lhsT=w1_sb[:, kk, mi * P:(mi + 1) * P],
                                        rhs=mT_sb[:, kk, :],
                                        start=(kk == 0), stop=(kk == KT - 1))
                            nc.vector.tensor_tensor(
                                out=h1T[:, hh2 * half:(hh2 + 1) * half, :],
                                in0=hp[:, :, :],
                                in1=b1c_sb[:, hh2 * half:(hh2 + 1) * half,
                                           None].to_broadcast((P, half, P)),
                                op=AX.add)
                        nc.scalar.activation(h1T[:, :, :], h1T[:, :, :],
                                             AF.Relu)
                        h2p = psum.tile([P, d], F32, tag="t1", bufs=2)
                        for mi in range(NMI):
                            nc.tensor.matmul(h2p[:, :], lhsT=h1T[:, mi, :],
                                             rhs=w2_sb[:, mi, :],
                                             start=(mi == 0),
                                             stop=(mi == NMI - 1))
                        nc.scalar.activation(g_grp[:, wi, 0:d], h2p[:, :],
                                             AF.Copy)

                    # ---- h + b2 + resid + LN2
                    nc.vector.tensor_tensor(
                        out=g_grp[:, :gn, 0:d], in0=g_grp[:, :gn, 0:d],
                        in1=b2_b[:, None, :].to_broadcast((P, gn, d)),
                        op=AX.add)
                    nc.vector.tensor_tensor(out=m_sb[:, gsl, :],
                                            in0=m_sb[:, gsl, :],
                                            in1=g_grp[:, :gn, 0:d],
                                            op=AX.add)
                    _layernorm_batched(nc, stats, m_sb[:, gsl, :],
                                       g_grp[:, :gn, 0:d], ln2g_b, ln2b_b,
                                       eps_t, gn, d)

            # ---------- output ----------
            for w in range(NW):
                nc.sync.dma_start(out=out_d[w * P:w * P + wsz(w), :],
                                  in_=m_sb[:wsz(w), w, :])
    if legalize:
        _legalize_single_wait(nc)
    return nc


def _layernorm_batched(nc, stats, m_ap, scratch_ap, g_b, b_b, eps_t, nwin, d):
    """In-place LayerNorm over the feature axis of m_ap [P, nwin, d]."""
    ssum = stats.tile([P, nwin], F32, tag="ssum", bufs=2)
    nc.vector.tensor_reduce(out=ssum[:, :], in_=m_ap,
                            axis=mybir.AxisListType.X, op=AX.add)
    nc.vector.tensor_tensor(out=scratch_ap, in0=m_ap, in1=m_ap, op=AX.mult)
    ssq = stats.tile([P, nwin], F32, tag="ssq", bufs=2)
    nc.vector.tensor_reduce(out=ssq[:, :], in_=scratch_ap,
                            axis=mybir.AxisListType.X, op=AX.add)
    mu = stats.tile([P, nwin], F32, tag="mu", bufs=2)
    nc.vector.tensor_scalar(out=mu[:, :], in0=ssum[:, :], scalar1=1.0 / d,
                            scalar2=None, op0=AX.mult)
    var = stats.tile([P, nwin], F32, tag="var", bufs=2)
    nc.vector.tensor_scalar(out=var[:, :], in0=ssq[:, :], scalar1=1.0 / d,
                            scalar2=None, op0=AX.mult)
    mu2 = stats.tile([P, nwin], F32, tag="mu2", bufs=2)
    nc.vector.tensor_tensor(out=mu2[:, :], in0=mu[:, :], in1=mu[:, :],
                            op=AX.mult)
    nc.vector.tensor_tensor(out=var[:, :], in0=var[:, :], in1=mu2[:, :],
                            op=AX.subtract)
    nc.scalar.activation(var[:, :], var[:, :], AF.Sqrt, bias=eps_t[:, :1])
    rstd = stats.tile([P, nwin], F32, tag="rstd", bufs=2)
    nc.vector.reciprocal(rstd[:, :], var[:, :])
    nc.vector.tensor_tensor(out=m_ap, in0=m_ap,
                            in1=mu[:, :, None].to_broadcast((P, nwin, d)),
                            op=AX.subtract)
    nc.vector.tensor_tensor(out=m_ap, in0=m_ap,
                            in1=rstd[:, :, None].to_broadcast((P, nwin, d)),
                            op=AX.mult)
    nc.vector.tensor_tensor(out=m_ap, in0=m_ap,
                            in1=g_b[:, None, :].to_broadcast((P, nwin, d)),
                            op=AX.mult)
    nc.vector.tensor_tensor(out=m_ap, in0=m_ap,
                            in1=b_b[:, None, :].to_broadcast((P, nwin, d)),
                            op=AX.add)


# ---------------------------------------------------------------------------
# entry point
# ---------------------------------------------------------------------------

_LAST_RESULTS = {}


def kernel(**inputs):
    n_nodes = inputs["x"].shape[0]
    n_edges = inputs["edge_index"].shape[1]
    n_layers = inputs["ffn_w1"].shape[0]
    in_maps, cfg = _prep(inputs, n_nodes, n_edges, n_layers, NCORES)
    nc = build_nc(cfg)
    res = run_bass_kernel_spmd(
        nc, in_maps, list(range(NCORES)),
        trace=bool(int(os.environ.get("GAT_TRACE", "0"))),
    )
    _LAST_RESULTS["res"] = res
    out = np.concatenate([res.results[cc]["out_d"] for cc in range(NCORES)],
                         axis=0)
    return out


# revision 8
# speedup vs baseline: 1.0830x; 1.0390x over previous
"""GAT message-passing kernel for Trainium2, 8 NeuronCores.

Design (vs the straightforward implementation):
  - One-hot scatter matrices and their transposes are built on the host
    (layer-invariant), stored interleaved bf16 in DRAM (one tensor, one
    DMA per window).  Building them on-device with DVE is_equal
    broadcasts cost ~40ms of the original 97ms.
  - Per-edge a_dst comes from tiny PE matmuls (lhsT=ohT, rhs=adst window
    column) instead of thousands of small indirect DMAs.
  - The all-gathered projection table [x_proj | a_src] is bf16: halves
    the AllGather and per-edge gather traffic.  All big matmuls take
    bf16 inputs (1 cycle/row on PE vs 4 for fp32); PSUM accumulation,
    the residual stream, LayerNorms and the softmax division stay fp32.
  - Per-window gather tile counts are trimmed to the max over cores
    (SPMD-shared program) instead of the global max.
  - Edge-phase tiles are quad-buffered and the aed+a_dst add is issued
    before the gathers complete to shorten the post-gather chain.
"""

import math
import os

import numpy as np
import ml_dtypes

import concourse.bass as bass
import concourse.mybir as mybir
import concourse.tile as tile
from concourse.bass_utils import run_bass_kernel_spmd

# problem dims (hardcoded per contract)
N, E, D, H, ED, L = 50000, 800000, 256, 8, 16, 6
C = D // H  # 32
DFF = 4 * D  # 1024
NEG_SLOPE = 0.2
EPS = 1e-5
NCORES = 8
P = 128

F32 = mybir.dt.float32
BF16 = mybir.dt.bfloat16
I32 = mybir.dt.int32
AX = mybir.AluOpType
AF = mybir.ActivationFunctionType

NEG_BIG = -1.0e30  # padded-edge a_edge => exp -> 0
NPBF = ml_dtypes.bfloat16


# ---------------------------------------------------------------------------
# host-side preprocessing
# ---------------------------------------------------------------------------

def _prep(inputs, n_nodes, n_edges, n_layers, n_cores):
    """Builds per-core input maps + the compile-time config."""
    x = np.asarray(inputs["x"], np.float32)
    edge_index = np.asarray(inputs["edge_index"])
    edge_attr = np.asarray(inputs["edge_attr"], np.float32)
    W = np.asarray(inputs["W"], np.float32)
    att_src = np.asarray(inputs["att_src"], np.float32)
    att_dst = np.asarray(inputs["att_dst"], np.float32)
    att_edge = np.asarray(inputs["att_edge"], np.float32)
    W_edge = np.asarray(inputs["W_edge"], np.float32)
    gat_bias = np.asarray(inputs["bias"], np.float32)

    d = W.shape[0]
    h = att_src.shape[0]
    c = att_src.shape[1]

    nsh = n_nodes // n_cores
    assert nsh * n_cores == n_nodes
    nw = math.ceil(nsh / P)
    lw = nsh - (nw - 1) * P  # rows in last window

    # fold attention vectors into the projection:  m @ W_ext ->
    # [x_proj | a_src | a_dst]
    S_src = np.zeros((d, h), np.float32)
    S_dst = np.zeros((d, h), np.float32)
    for hh in range(h):
        S_src[hh * c:(hh + 1) * c, hh] = att_src[hh]
        S_dst[hh * c:(hh + 1) * c, hh] = att_dst[hh]
    W_ext = np.concatenate([W, W @ S_src, W @ S_dst], axis=1)  # [d, d+2h]

    # per-edge a_edge = sum_c (ea @ W_edge)[h,c] * att_edge[h,c] = ea @ V
    V = np.zeros((W_edge.shape[0], h), np.float32)
    for hh in range(h):
        V[:, hh] = W_edge[:, hh * c:(hh + 1) * c] @ att_edge[hh]
    src0 = edge_index[0].astype(np.int64)
    dst0 = edge_index[1].astype(np.int64)

    # self-loop edge_attr = mean of incoming real-edge attrs (PyG default)
    order = np.argsort(dst0, kind="stable")
    dst_s = dst0[order]
    src_s = src0[order]
    ea_sum = np.zeros((n_nodes, W_edge.shape[0]), np.float32)
    if n_edges > 0:
        starts = np.flatnonzero(np.r_[True, dst_s[1:] != dst_s[:-1]])
        sums = np.add.reduceat(edge_attr[order], starts, axis=0)
        ea_sum[dst_s[starts]] = sums
    deg = np.bincount(dst0, minlength=n_nodes).astype(np.float32)
    ea_mean = ea_sum / np.maximum(deg, 1.0)[:, None]

    a_edge_real = (edge_attr @ V).astype(np.float32)[order]  # dst-sorted
    a_edge_self = (ea_mean @ V).astype(np.float32)

    # build per-core padded edge structure
    counts = np.zeros((n_cores, nw), np.int64)
    core_all = dst_s // nsh
    lw_all = (dst_s - core_all * nsh) // P
    np.add.at(counts, (core_all, lw_all), 1)
    node_ids = np.arange(n_nodes, dtype=np.int64)
    sc = node_ids // nsh
    slw = (node_ids - sc * nsh) // P
    np.add.at(counts, (sc, slw), 1)
    T = int(math.ceil(counts.max() / P))

    src_arr = np.zeros((n_cores, nw, P, T), np.int32)
    dl_arr = np.zeros((n_cores, nw, P, T), np.int32)
    aed_arr = np.full((n_cores, nw, P, T, h), NEG_BIG, np.float32)

    for cc in range(n_cores):
        base = cc * nsh
        lo = np.searchsorted(dst_s, base)
        hi = np.searchsorted(dst_s, base + nsh)
        cdst = dst_s[lo:hi] - base
        csrc = src_s[lo:hi]
        caed = a_edge_real[lo:hi]
        cw = cdst // P
        wstarts = np.searchsorted(cw, np.arange(nw))
        wends = np.searchsorted(cw, np.arange(nw) + 1)
        for w in range(nw):
            wsz = lw if w == nw - 1 else P
            s, e = wstarts[w], wends[w]
            self_nodes = base + w * P + np.arange(wsz)
            # self-loop block first, padded to exactly P slots (tile 0) so
            # the device can service it with one contiguous shard DMA
            pad = P - wsz
            srcs = np.concatenate([self_nodes,
                                   np.zeros(pad, np.int64), csrc[s:e]])
            dls = np.concatenate([np.arange(wsz, dtype=np.int64),
                                  np.zeros(pad, np.int64),
                                  cdst[s:e] - w * P])
            aeds = np.concatenate([a_edge_self[self_nodes],
                                   np.full((pad, h), NEG_BIG, np.float32),
                                   caed[s:e]], axis=0)
            k = srcs.shape[0]
            assert k <= P * T
            pj = np.arange(k) % P
            tj = np.arange(k) // P
            src_arr[cc, w, pj, tj] = srcs
            dl_arr[cc, w, pj, tj] = dls
            aed_arr[cc, w, pj, tj] = aeds

    # ffn / ln weights
    w1 = np.asarray(inputs["ffn_w1"], np.float32)
    b1 = np.asarray(inputs["ffn_b1"], np.float32)
    w2 = np.asarray(inputs["ffn_w2"], np.float32)
    b2 = np.ascontiguousarray(np.asarray(inputs["ffn_b2"], np.float32))
    ln1g = np.ascontiguousarray(np.asarray(inputs["ln1_g"], np.float32))
    ln1b = np.ascontiguousarray(np.asarray(inputs["ln1_b"], np.float32))
    ln2g = np.ascontiguousarray(np.asarray(inputs["ln2_g"], np.float32))
    ln2b = np.ascontiguousarray(np.asarray(inputs["ln2_b"], np.float32))
    nmi = w1.shape[2] // P
    b1c = np.ascontiguousarray(
        b1.reshape(n_layers, nmi, P).transpose(0, 2, 1))

    # per-window tile counts: max over cores so the SPMD program is shared
    # (+ the self-loop padding in the last window's tile 0)
    counts_pad = counts.copy()
    counts_pad[:, nw - 1] += P - lw
    Tw = np.maximum(1, np.ceil(counts_pad.max(axis=0) / P)).astype(int)

    # one-hot scatter matrices, bf16, layer-invariant, interleaved with
    # their transposes: ohh[cc, w, p, t, 0, :]=oh, [..., 1, :]=ohT
    eye = np.arange(P, dtype=np.int32)
    oh_all = (dl_arr[..., None] == eye).astype(NPBF)       # [cc,nw,P,T,P]
    ohT_all = oh_all.transpose(0, 1, 4, 3, 2)
    ohh_all = np.ascontiguousarray(
        np.stack([oh_all, ohT_all], axis=4))               # [cc,nw,P,T,2,P]
    del oh_all, ohT_all

    w1b = np.ascontiguousarray(w1.astype(NPBF))
    w2b = np.ascontiguousarray(w2.astype(NPBF))
    wextb = np.ascontiguousarray(W_ext.astype(NPBF))
    aedb = np.ascontiguousarray(aed_arr.astype(NPBF))

    in_maps = []
    for cc in range(n_cores):
        in_maps.append({
            "x0": np.ascontiguousarray(x[cc * nsh:(cc + 1) * nsh]),
            "w_ext": wextb,
            "gat_bias": gat_bias.copy(),
            "w1_d": w1b, "b1c_d": b1c, "w2_d": w2b, "b2_d": b2,
            "ln1g_d": ln1g, "ln1b_d": ln1b,
            "ln2g_d": ln2g, "ln2b_d": ln2b,
            "src_d": np.ascontiguousarray(src_arr[cc]),
            "aed_d": aedb[cc],
            "ohh_d": ohh_all[cc],
        })
    cfg = dict(n_nodes=n_nodes, nsh=nsh, nw=nw, lw=lw, T=T,
               n_layers=n_layers, n_cores=n_cores, d=d, h=h,
               dff=w1.shape[2], Tw=[int(t) for t in Tw])
    return in_maps, cfg


# ---------------------------------------------------------------------------
# device kernel
# ---------------------------------------------------------------------------

def _legalize_single_wait(nc):
    """This walrus build allows at most one sync wait per instruction.

    Split extra waits onto standalone EventSemaphore instructions right
    before the owner (same engine => identical semantics).
    """
    def fix(blocks):
        n = 0
        for blk in blocks:
            newl = []
            for inst in list(blk.instructions):
                si = getattr(inst, "sync_info", None)
                ow = list(si.on_wait) if (si is not None and si.on_wait) else []
                if len(ow) > 1:
                    for j, wt in enumerate(ow[:-1]):
                        newl.append(mybir.InstEventSemaphore(
                            name=f"{inst.name}wf{j}",
                            sync_info=mybir.SyncInfo(on_wait=[wt],
                                                     on_update=[]),
                            engine=inst.engine,
                        ))
                    inst.sync_info = mybir.SyncInfo(
                        on_wait=[ow[-1]], on_update=list(si.on_update))
                    n += 1
                newl.append(inst)
            blk.instructions = newl
            subs = list(blk.blocks) if getattr(blk, "blocks", None) else []
            if subs:
                n += fix(subs)
        return n

    for f in nc.m.functions:
        fix(list(f.blocks))


def build_nc(cfg, legalize=True):
    n_nodes = cfg["n_nodes"]
    nsh = cfg["nsh"]
    NW = cfg["nw"]
    LW = cfg["lw"]
    T = cfg["T"]
    LAYERS = cfg["n_layers"]
    n_cores = cfg["n_cores"]
    d = cfg["d"]
    h = cfg["h"]
    dff = cfg["dff"]
    KT = d // P          # K-tiles for D-contraction (2)
    NMI = dff // P       # M-tiles for dff (8)
    TBL = d + h          # 264: x_proj | a_src
    EXT = d + 2 * h      # 272: x_proj | a_src | a_dst

    abl = set((os.environ.get("GAT_ABL") or "").split(","))
    Tw = cfg.get("Tw") or [T] * NW

    nc = bass.Bass("TRN2", target_bir_lowering=False, debug=False,
                   num_devices=n_cores,
                   dynamic_dma_scratch_size=int(
                       os.environ.get("GAT_DMASCRATCH", "16384")))

    x0 = nc.dram_tensor("x0", [nsh, d], F32, kind="ExternalInput")
    w_ext = nc.dram_tensor("w_ext", [d, EXT], BF16, kind="ExternalInput")
    gat_bias = nc.dram_tensor("gat_bias", [d], F32, kind="ExternalInput")
    w1_d = nc.dram_tensor("w1_d", [LAYERS, d, dff], BF16,
                          kind="ExternalInput")
    b1c_d = nc.dram_tensor("b1c_d", [LAYERS, P, NMI], F32,
                           kind="ExternalInput")
    w2_d = nc.dram_tensor("w2_d", [LAYERS, dff, d], BF16,
                          kind="ExternalInput")
    b2_d = nc.dram_tensor("b2_d", [LAYERS, d], F32, kind="ExternalInput")
    ln1g_d = nc.dram_tensor("ln1g_d", [LAYERS, d], F32, kind="ExternalInput")
    ln1b_d = nc.dram_tensor("ln1b_d", [LAYERS, d], F32, kind="ExternalInput")
    ln2g_d = nc.dram_tensor("ln2g_d", [LAYERS, d], F32, kind="ExternalInput")
    ln2b_d = nc.dram_tensor("ln2b_d", [LAYERS, d], F32, kind="ExternalInput")
    src_d = nc.dram_tensor("src_d", [NW, P, T], I32, kind="ExternalInput")
    aed_d = nc.dram_tensor("aed_d", [NW, P, T, h], BF16,
                           kind="ExternalInput")
    ohh_d = nc.dram_tensor("ohh_d", [NW, P, T, 2, P], BF16,
                           kind="ExternalInput")
    out_d = nc.dram_tensor("out_d", [nsh, d], F32, kind="ExternalOutput")

    G = min(7, NW)  # windows per group (epilogue/FFN batching granularity)
    groups = [(g0, min(g0 + G, NW) - g0) for g0 in range(0, NW, G)]

    def wsz(w):
        return LW if w == NW - 1 else P

    with tile.TileContext(nc) as tc:
        with (
            tc.tile_pool(name="const", bufs=1) as const,
            tc.tile_pool(name="wpool", bufs=2) as wpool,
            tc.tile_pool(name="big", bufs=1) as big,
            tc.tile_pool(name="work", bufs=2) as work,
            tc.tile_pool(name="edge", bufs=4) as edge,
            tc.tile_pool(name="small", bufs=2) as small,
            tc.tile_pool(name="stats", bufs=1) as stats,
            tc.tile_pool(name="psum", bufs=1, space="PSUM") as psum,
            tc.tile_pool(name="dram", bufs=1, space="DRAM") as dram,
        ):
            # ---------- constants ----------
            if "reset" in (os.environ.get("GAT_OPT") or ""):
                nc.gpsimd.dma_reset()
            from concourse.masks import make_identity
            identf = const.tile([P, P], F32)
            make_identity(nc, identf[:, :])
            wext_sb = const.tile([P, KT, EXT], BF16)
            nc.sync.dma_start(
                out=wext_sb[:, :, :],
                in_=w_ext[:, :].rearrange("(kk p) c -> p kk c", p=P))
            bias_b = const.tile([P, d], F32)
            nc.sync.dma_start(out=bias_b[:, :],
                              in_=gat_bias[None, :].to_broadcast((P, d)))
            eps_t = const.tile([P, 1], F32)
            nc.vector.memset(eps_t[:, :], EPS)

            # ---------- persistent node state ----------
            m_sb = big.tile([P, NW, d], F32)
            if LW < P:
                nc.vector.memset(m_sb[:, NW - 1, :], 0.0)
            for w in range(NW):
                nc.sync.dma_start(out=m_sb[:wsz(w), w, :],
                                  in_=x0[w * P:w * P + wsz(w), :])
            adst_sb = big.tile([P, NW, h], BF16)

            REP = int(os.environ.get("GAT_REPEAT", "1"))
            _flat = [] if "empty" in abl else \
                [ly for _ in range(REP) for ly in range(LAYERS)]
            if "dummyag" in abl:
                dag_in = dram.tile([P, 4], F32, tag="dagi")
                dag_out = dram.tile([P * n_cores, 4], F32, tag="dago",
                                    addr_space="Shared")
                z4 = const.tile([P, 4], F32)
                nc.vector.memset(z4[:, :], 1.0)
                nc.sync.dma_start(out=dag_in[:, :], in_=z4[:, :])
                nc.gpsimd.collective_compute(
                    "AllGather", AX.bypass,
                    replica_groups=[list(range(n_cores))],
                    ins=[dag_in.opt()], outs=[dag_out.opt()])
            for layer in _flat:
                # ---------- per-layer weights ----------
                w1_sb = wpool.tile([P, KT, dff], BF16, tag="w1")
                nc.sync.dma_start(
                    out=w1_sb[:, :, :],
                    in_=w1_d[layer, :, :].rearrange("(kk p) f -> p kk f", p=P))
                w2_sb = wpool.tile([P, NMI, d], BF16, tag="w2")
                nc.sync.dma_start(
                    out=w2_sb[:, :, :],
                    in_=w2_d[layer, :, :].rearrange("(kk p) f -> p kk f", p=P))
                b1c_sb = wpool.tile([P, NMI], F32, tag="b1c")
                nc.sync.dma_start(out=b1c_sb[:, :], in_=b1c_d[layer, :, :])
                b2_b = wpool.tile([P, d], F32, tag="b2")
                nc.sync.dma_start(
                    out=b2_b[:, :],
                    in_=b2_d[layer:layer + 1, :].to_broadcast((P, d)))
                ln1g_b = wpool.tile([P, d], F32, tag="ln1g")
                nc.sync.dma_start(
                    out=ln1g_b[:, :],
                    in_=ln1g_d[layer:layer + 1, :].to_broadcast((P, d)))
                ln1b_b = wpool.tile([P, d], F32, tag="ln1b")
                nc.sync.dma_start(
                    out=ln1b_b[:, :],
                    in_=ln1b_d[layer:layer + 1, :].to_broadcast((P, d)))
                ln2g_b = wpool.tile([P, d], F32, tag="ln2g")
                nc.sync.dma_start(
                    out=ln2g_b[:, :],
                    in_=ln2g_d[layer:layer + 1, :].to_broadcast((P, d)))
                ln2b_b = wpool.tile([P, d], F32, tag="ln2b")
                nc.sync.dma_start(
                    out=ln2b_b[:, :],
                    in_=ln2b_d[layer:layer + 1, :].to_broadcast((P, d)))

                shard_t = dram.tile([nsh, TBL], BF16, tag="shard")
                table_t = dram.tile([n_nodes, TBL], BF16, tag="table",
                                    addr_space="Shared")

                # ---------- phase A: x_proj | a_src | a_dst ----------
                for w in range(NW):
                    mT_sb = work.tile([P, KT, P], BF16, tag="mT")
                    for kk in range(KT):
                        tp = psum.tile([P, P], F32, tag="t1", bufs=2)
                        nc.tensor.transpose(
                            tp[:, :], m_sb[:, w, kk * P:(kk + 1) * P],
                            identf[:, :])
                        nc.scalar.activation(mT_sb[:, kk, :], tp[:, :],
                                             AF.Copy)
                    pj = psum.tile([P, EXT], F32, tag="t2", bufs=3)
                    for kk in range(KT):
                        nc.tensor.matmul(pj[:, :], lhsT=mT_sb[:, kk, :],
                                         rhs=wext_sb[:, kk, :],
                                         start=(kk == 0), stop=(kk == KT - 1))
                    prj = work.tile([P, EXT], BF16, tag="prj")
                    nc.scalar.activation(prj[:, :], pj[:, :], AF.Copy)
                    nc.sync.dma_start(out=shard_t[w * P:w * P + wsz(w), :],
                                      in_=prj[:wsz(w), :TBL])
                    nc.vector.tensor_copy(adst_sb[:, w, :], prj[:, TBL:EXT])

                # ---------- all-gather the projection table ----------
                if "noag" not in abl:
                    nc.gpsimd.collective_compute(
                        "AllGather",
                        AX.bypass,
                        replica_groups=[list(range(n_cores))],
                        ins=[shard_t.opt()],
                        outs=[table_t.opt()],
                    )

                # ---------- grouped phases B (edges) + C (FFN) ----------
                for g0, gn in groups:
                    gsl = slice(g0, g0 + gn)
                    g_grp = work.tile([P, G, TBL], F32, tag="ggrp")
                    # group metadata in one DMA each
                    idx_g = small.tile([P, G, T], I32, tag="idx")
                    nc.sync.dma_start(out=idx_g[:, :gn, :],
                                      in_=src_d[gsl, :, :].rearrange(
                                          "g p t -> p g t"))
                    aed_g = work.tile([P, G, T, h], BF16, tag="aed")
                    nc.sync.dma_start(out=aed_g[:, :gn, :, :],
                                      in_=aed_d[gsl, :, :, :].rearrange(
                                          "g p t hh -> p g t hh"))
                    for wi in range(gn):
                        w = g0 + wi
                        tw = Tw[w]
                        ohh_w = edge.tile([P, T, 2, P], BF16, tag="ohh")
                        nc.sync.dma_start(out=ohh_w[:, :tw, :, :],
                                          in_=ohh_d[w, :, :tw, :, :])
                        gat_w = edge.tile([P, T, TBL], BF16, tag="gat")
                        if "nogat" in abl:
                            nc.vector.memset(gat_w[:, 0, :], 0.5)
                        else:
                            # tile 0 = this window's self-loops: contiguous
                            # rows of the LOCAL shard (no AllGather or
                            # indirect-DMA dependency)
                            if wsz(w) < P:
                                nc.vector.memset(gat_w[:, 0, :], 0.0)
                            nc.sync.dma_start(
                                out=gat_w[:wsz(w), 0, :],
                                in_=shard_t[w * P:w * P + wsz(w), :])
                            for t in range(1, tw):
                                nc.gpsimd.indirect_dma_start(
                                    out=gat_w[:, t, :],
                                    out_offset=None,
                                    in_=table_t[:, :],
                                    in_offset=bass.IndirectOffsetOnAxis(
                                        ap=idx_g[:, wi, t:t + 1], axis=0),
                                )
                        # per-edge a_dst via tiny matmuls: [edge,h] per t
                        adxp = psum.tile([P, T * h], F32, tag="t3", bufs=2)
                        for t in range(tw):
                            nc.tensor.matmul(adxp[:, t * h:(t + 1) * h],
                                             lhsT=ohh_w[:, t, 1, :],
                                             rhs=adst_sb[:, w, :],
                                             start=True, stop=True)
                        adx_w = small.tile([P, T, h], BF16, tag="adx")
                        nc.scalar.activation(
                            adx_w[:, :tw, :],
                            adxp[:, :tw * h].rearrange(
                                "p (t hh) -> p t hh", hh=h),
                            AF.Copy)
                        # alpha = a_src[src]+a_dst[dst]+a_edge; lrelu; exp
                        # (aed + a_dst first: no gather dependency, so it
                        # overlaps with the indirect gathers)
                        alpha = small.tile([P, T, h], BF16, tag="alpha")
                        nc.vector.tensor_tensor(out=alpha[:, :tw, :],
                                                in0=aed_g[:, wi, :tw, :],
                                                in1=adx_w[:, :tw, :],
                                                op=AX.add)
                        nc.vector.tensor_tensor(out=alpha[:, :tw, :],
                                                in0=alpha[:, :tw, :],
                                                in1=gat_w[:, :tw, d:TBL],
                                                op=AX.add)
                        nc.vector.tensor_scalar(out=adx_w[:, :tw, :],
                                                in0=alpha[:, :tw, :],
                                                scalar1=NEG_SLOPE,
                                                scalar2=None, op0=AX.mult)
                        nc.vector.tensor_tensor(out=alpha[:, :tw, :],
                                                in0=alpha[:, :tw, :],
                                                in1=adx_w[:, :tw, :],
                                                op=AX.max)
                        nc.scalar.activation(gat_w[:, :tw, d:TBL],
                                             alpha[:, :tw, :], AF.Exp)
                        # messages: x_proj *= ex (broadcast over channels)
                        nc.vector.tensor_tensor(
                            out=gat_w[:, :tw, 0:d].rearrange(
                                "p t (hh c) -> p t hh c", c=C),
                            in0=gat_w[:, :tw, 0:d].rearrange(
                                "p t (hh c) -> p t hh c", c=C),
                            in1=gat_w[:, :tw, d:TBL][:, :, :,
                                                     None].to_broadcast(
                                (P, tw, h, C)),
                            op=AX.mult)
                        # accumulate  [window, x_sum | ex_sum]
                        acc = psum.tile([P, TBL], F32, tag="t2", bufs=3)
                        nts = 1 if "noacc" in abl else tw
                        for t in range(nts):
                            nc.tensor.matmul(acc[:, :], lhsT=ohh_w[:, t, 0, :],
                                             rhs=gat_w[:, t, :],
                                             start=(t == 0),
                                             stop=(t == nts - 1))
                        nc.scalar.activation(g_grp[:, wi, :], acc[:, :],
                                             AF.Copy)

                    # ---- group epilogue: softmax div + bias + resid + LN1
                    den = g_grp[:, :gn, d:TBL]
                    nc.vector.tensor_scalar(out=den, in0=den, scalar1=1e-30,
                                            scalar2=None, op0=AX.max)
                    rec = stats.tile([P, G, h], F32, tag="rec", bufs=2)
                    nc.vector.reciprocal(rec[:, :gn, :], den)
                    nc.vector.tensor_tensor(
                        out=g_grp[:, :gn, 0:d].rearrange(
                            "p w (hh c) -> p w hh c", c=C),
                        in0=g_grp[:, :gn, 0:d].rearrange(
                            "p w (hh c) -> p w hh c", c=C),
                        in1=rec[:, :gn, :, None].to_broadcast((P, gn, h, C)),
                        op=AX.mult)
                    nc.vector.tensor_tensor(
                        out=g_grp[:, :gn, 0:d], in0=g_grp[:, :gn, 0:d],
                        in1=bias_b[:, None, :].to_broadcast((P, gn, d)),
                        op=AX.add)
                    nc.vector.tensor_tensor(out=m_sb[:, gsl, :],
                                            in0=m_sb[:, gsl, :],
                                            in1=g_grp[:, :gn, 0:d],
                                            op=AX.add)
                    _layernorm_batched(nc, stats, m_sb[:, gsl, :],
                                       g_grp[:, :gn, 0:d], ln1g_b, ln1b_b,
                                       eps_t, gn, d)

                    # ---- FFN for the group's windows
                    for wi in range(gn):
                        w = g0 + wi
                        mT_sb = work.tile([P, KT, P], BF16, tag="mT")
                        for kk in range(KT):
                            tp = psum.tile([P, P], F32, tag="t1", bufs=2)
                            nc.tensor.transpose(
                                tp[:, :], m_sb[:, w, kk * P:(kk + 1) * P],
                                identf[:, :])
                            nc.scalar.activation(mT_sb[:, kk, :], tp[:, :],
                                                 AF.Copy)
                        h1T = work.tile([P, NMI, P], BF16, tag="h1T")
                        half = NMI // 2
                        for hh2 in range(2):
                            tag = "t2" if hh2 == 0 else "t3"
                            hp = psum.tile([P, half, P], F32, tag=tag,
                                           bufs=3 if hh2 == 0 else 2)
                            for sl in range(half):
                                mi = hh2 * half + sl
                                for kk in range(KT):
                                    nc.tensor.matmul(
                                        hp[:, sl, :],
                                        lhsT=w1_sb[:, kk, mi * P:(mi + 1) * P],
                                        rhs=mT_sb[:, kk, :],
                                        start=(kk == 0), stop=(kk == KT - 1))
                            nc.vector.tensor_tensor(
                                out=h1T[:, hh2 * half:(hh2 + 1) * half, :],
                                in0=hp[:, :, :],
                                in1=b1c_sb[:, hh2 * half:(hh2 + 1) * half,
                                           None].to_broadcast((P, half, P)),
                                op=AX.add)
                        nc.scalar.activation(h1T[:, :, :], h1T[:, :, :],
                                             AF.Relu)
                        h2p = psum.tile([P, d], F32, tag="t1", bufs=2)
                        for mi in range(NMI):
                            nc.tensor.matmul(h2p[:, :], lhsT=h1T[:, mi, :],
                                             rhs=w2_sb[:, mi, :],
                                             start=(mi == 0),
                                             stop=(mi == NMI - 1))
                        nc.scalar.activation(g_grp[:, wi, 0:d], h2p[:, :],
                                             AF.Copy)

                    # ---- h + b2 + resid + LN2
                    nc.vector.tensor_tensor(
                        out=g_grp[:, :gn, 0:d], in0=g_grp[:, :gn, 0:d],
                        in1=b2_b[:, None, :].to_broadcast((P, gn, d)),
                        op=AX.add)
                    nc.vector.tensor_tensor(out=m_sb[:, gsl, :],
                                            in0=m_sb[:, gsl, :],
                                            in1=g_grp[:, :gn, 0:d],
                                            op=AX.add)
                    _layernorm_batched(nc, stats, m_sb[:, gsl, :],
                                       g_grp[:, :gn, 0:d], ln2g_b, ln2b_b,
                                       eps_t, gn, d)

            # ---------- output ----------
            for w in range(NW):
                nc.sync.dma_start(out=out_d[w * P:w * P + wsz(w), :],
                                  in_=m_sb[:wsz(w), w, :])
    if legalize:
        _legalize_single_wait(nc)
    return nc


def _layernorm_batched(nc, stats, m_ap, scratch_ap, g_b, b_b, eps_t, nwin, d):
    """In-place LayerNorm over the feature axis of m_ap [P, nwin, d]."""
    ssum = stats.tile([P, nwin], F32, tag="ssum", bufs=2)
    nc.vector.tensor_reduce(out=ssum[:, :], in_=m_ap,
                            axis=mybir.AxisListType.X, op=AX.add)
    nc.vector.tensor_tensor(out=scratch_ap, in0=m_ap, in1=m_ap, op=AX.mult)
    ssq = stats.tile([P, nwin], F32, tag="ssq", bufs=2)
    nc.vector.tensor_reduce(out=ssq[:, :], in_=scratch_ap,
                            axis=mybir.AxisListType.X, op=AX.add)
    mu = stats.tile([P, nwin], F32, tag="mu", bufs=2)
    nc.vector.tensor_scalar(out=mu[:, :], in0=ssum[:, :], scalar1=1.0 / d,
                            scalar2=None, op0=AX.mult)
    var = stats.tile([P, nwin], F32, tag="var", bufs=2)
    nc.vector.tensor_scalar(out=var[:, :], in0=ssq[:, :], scalar1=1.0 / d,
                            scalar2=None, op0=AX.mult)
    mu2 = stats.tile([P, nwin], F32, tag="mu2", bufs=2)
    nc.vector.tensor_tensor(out=mu2[:, :], in0=mu[:, :], in1=mu[:, :],
                            op=AX.mult)
    nc.vector.tensor_tensor(out=var[:, :], in0=var[:, :], in1=mu2[:, :],
                            op=AX.subtract)
    nc.scalar.activation(var[:, :], var[:, :], AF.Sqrt, bias=eps_t[:, :1])
    rstd = stats.tile([P, nwin], F32, tag="rstd", bufs=2)
    nc.vector.reciprocal(rstd[:, :], var[:, :])
    nc.vector.tensor_tensor(out=m_ap, in0=m_ap,
                            in1=mu[:, :, None].to_broadcast((P, nwin, d)),
                            op=AX.subtract)
    nc.vector.tensor_tensor(out=m_ap, in0=m_ap,
                            in1=rstd[:, :, None].to_broadcast((P, nwin, d)),
                            op=AX.mult)
    nc.vector.tensor_tensor(out=m_ap, in0=m_ap,
                            in1=g_b[:, None, :].to_broadcast((P, nwin, d)),
                            op=AX.mult)
    nc.vector.tensor_tensor(out=m_ap, in0=m_ap,
                            in1=b_b[:, None, :].to_broadcast((P, nwin, d)),
                            op=AX.add)


# ---------------------------------------------------------------------------
# entry point
# ---------------------------------------------------------------------------

_LAST_RESULTS = {}


def kernel(**inputs):
    n_nodes = inputs["x"].shape[0]
    n_edges = inputs["edge_index"].shape[1]
    n_layers = inputs["ffn_w1"].shape[0]
    in_maps, cfg = _prep(inputs, n_nodes, n_edges, n_layers, NCORES)
    nc = build_nc(cfg)
    res = run_bass_kernel_spmd(
        nc, in_maps, list(range(NCORES)),
        trace=bool(int(os.environ.get("GAT_TRACE", "0"))),
    )
    _LAST_RESULTS["res"] = res
    out = np.concatenate([res.results[cc]["out_d"] for cc in range(NCORES)],
                         axis=0)
    return out
